# revision 7
# baseline (speedup 1.0000x reference)
"""Trainium2 Bass kernel for AEDiscriminator: 2-layer GRU + 2-layer LSTM stacks
with length masking, concat + dense head.

Sharding (default mode "p2"): model-split x batch-split. The GRU stack runs on
cores 0-3 and the LSTM stack on cores 4-7, each group data-parallel over 16
examples/core. The two NEFFs are dispatched asynchronously on disjoint device
subsets. No inter-core communication: the dense head decomposes as
o1 @ W[:256] + o2 @ W[256:], summed on the host. Mode "p1" (K_MODE=p1) is a
single-graph fallback: all four layers on every core, 8 examples/core.

Per-core layout: features/gates on SBUF partitions (128-row tiles), batch on
the free dimension. Recurrent matmuls run weights-stationary (bf16):
out[gates, batch] = U[k, gates].T @ h[k, batch]. Layer-0 input projections
(x @ W + b) are precomputed for all timesteps as large matmuls in a prologue;
the T=512 recurrent loop is a hardware loop (For_i) with an 8-step unrolled
body and register-indexed access patterns; masking is a single copy_predicated
per state tensor. Layer-1 U-products are computed into separate PSUM regions
early (they need only h1(t-1)) so the PE never queue-stalls behind W-products
that wait on h0(t); ScalarE evacuates them. Per-step time is within ~15% of
the PE weight-load floor (~107 ns LDWEIGHTS per 128x128 tile; the toolchain's
walrus has --enable-ldw-opt=false, so fast-weight-load is unavailable).
"""

import os
from contextlib import ExitStack

import numpy as np
import ml_dtypes

import concourse.bass as bass
import concourse.tile as tile
from concourse import bacc, mybir
from concourse.bass_utils import run_bass_kernel_spmd
from concourse import bass_utils as _bu

if os.environ.get("K_LDWOPT", "0") == "1" and not getattr(_bu, "_ldw_patched", False):
    _orig_run_command = _bu.run_command

    def _patched_run_command(cmd, *a, **kw):
        cmd = [c.replace("--enable-ldw-opt=false", "--enable-ldw-opt=true")
               if isinstance(c, str) else c for c in cmd]
        return _orig_run_command(cmd, *a, **kw)

    _bu.run_command = _patched_run_command
    _bu._ldw_patched = True

BF16 = mybir.dt.bfloat16
FP8 = mybir.dt.float8e4
WSCALE = 16.0
F32 = mybir.dt.float32
U8 = mybir.dt.uint8
AF = mybir.ActivationFunctionType
OP = mybir.AluOpType
ds = bass.ds

B, T, D, H = 64, 512, 96, 256
NCORES = 8
B_LOC = B // NCORES          # 8 examples per core
UNROLL = int(os.environ.get("K_UNROLL", "8"))
NG_G, NG_L = 6, 8            # gate tiles of 128: GRU 768, LSTM 1024
CH = 512                     # free-dim chunk for bulk matmuls


def build_nc(b=B_LOC, with_gru=True, with_lstm=True, unroll=UNROLL,
             zero_bias=False, time_mult=1, fp8=False):
    nc = bacc.Bacc()
    TB = T * b

    xT = nc.declare_dram_parameter("xT", [D, TB], BF16, isOutput=False)
    mk = nc.declare_dram_parameter("mask", [128, T * 2 * b], U8, isOutput=False)
    y_ext = nc.declare_dram_parameter("y", [1, TB], F32, isOutput=True)

    def param(name, shape, dt=BF16):
        return nc.declare_dram_parameter(name, shape, dt, isOutput=False)

    WDT = FP8 if fp8 else BF16
    if with_gru:
        gW0 = param("gW0", [D, 768])
        gU0 = param("gU0", [128, 2 * 768], WDT)
        gW1 = param("gW1", [128, 2 * 768], WDT)
        gU1 = param("gU1", [128, 2 * 768], WDT)
        bxg = param("bxg", [128, NG_G], F32)        # x-proj bias per m-tile
        bzr1 = param("bzr1", [128, 4 * b], F32)     # L1 (bi+br) for z,r
        bxh1 = param("bxh1", [128, 2 * b], F32)     # L1 bi_h
        brh1 = param("brh1", [128, 2 * b], F32)     # L1 br_h
        brh0 = param("brh0", [128, 2 * b], F32)     # L0 br_h
    if with_lstm:
        lW0 = param("lW0", [D, 1024])
        lU0 = param("lU0", [128, 2 * 1024], WDT)
        lW1 = param("lW1", [128, 2 * 1024], WDT)
        lU1 = param("lU1", [128, 2 * 1024], WDT)
        bxl = param("bxl", [128, NG_L], F32)        # x-proj bias per m-tile
        bl1 = param("bl1", [128, NG_L * b], F32)    # L1 bias, broadcast over b
    n_head_k = (2 if with_gru else 0) + (2 if with_lstm else 0)
    wo = param("wo", [128, n_head_k])

    trace_sim = os.environ.get("K_TRACE", "0") == "1"
    with tile.TileContext(nc, trace_sim=trace_sim) as tc, ExitStack() as ctx:
        pool = ctx.enter_context(tc.tile_pool(name="main", bufs=1))
        stg = ctx.enter_context(tc.tile_pool(name="stg", bufs=3))
        tmp = ctx.enter_context(tc.tile_pool(name="tmp", bufs=3))
        psx = ctx.enter_context(tc.tile_pool(name="psx", bufs=2, space="PSUM"))
        psr_bufs = 1 if (with_gru and with_lstm) else 2
        psr = ctx.enter_context(tc.tile_pool(name="psr", bufs=psr_bufs, space="PSUM"))

        # ---- persistent SBUF tensors -----------------------------------
        mk_sb = pool.tile([128, T * 2 * b], U8)
        nc.sync.dma_start(mk_sb[:], mk[:])
        wo_sb = pool.tile([128, n_head_k], BF16)
        nc.sync.dma_start(wo_sb[:], wo[:])

        def load(p, shape, dt=BF16):
            t_ = pool.tile(shape, dt, tag=f"w_{p.name}")
            nc.sync.dma_start(t_[:], p[:])
            return t_

        if with_gru:
            gW0s = load(gW0, [D, 768])
            gU0s = load(gU0, [128, 2 * 768], WDT)
            gW1s = load(gW1, [128, 2 * 768], WDT)
            gU1s = load(gU1, [128, 2 * 768], WDT)
            bxgs = load(bxg, [128, NG_G], F32)
            bzr1s = load(bzr1, [128, 4 * b], F32)
            bxh1s = load(bxh1, [128, 2 * b], F32)
            brh1s = load(brh1, [128, 2 * b], F32)
            brh0s = load(brh0, [128, 2 * b], F32)
            gx_g = pool.tile([128, NG_G, TB], BF16)   # precomputed x-proj GRU L0
            o1 = pool.tile([128, 2, TB], BF16)        # GRU L1 output history
            hG0 = pool.tile([128, 2 * b], BF16)
            hG1 = pool.tile([128, 2 * b], BF16)
            nc.vector.memset(hG0[:], 0.0)
            nc.vector.memset(hG1[:], 0.0)
        if with_lstm:
            lW0s = load(lW0, [D, 1024])
            lU0s = load(lU0, [128, 2 * 1024], WDT)
            lW1s = load(lW1, [128, 2 * 1024], WDT)
            lU1s = load(lU1, [128, 2 * 1024], WDT)
            bxls = load(bxl, [128, NG_L], F32)
            bl1s = load(bl1, [128, NG_L * b], F32)
            gx_l = pool.tile([128, NG_L, TB], BF16)   # precomputed x-proj LSTM L0
            o2 = pool.tile([128, 2, TB], BF16)        # LSTM L1 output history
            hL0 = pool.tile([128, 2 * b], BF16)
            hL1 = pool.tile([128, 2 * b], BF16)
            cL0 = pool.tile([128, 2 * b], F32)
            cL1 = pool.tile([128, 2 * b], F32)
            for t_ in (hL0, hL1, cL0, cL1):
                nc.vector.memset(t_[:], 0.0)

        # ---- prologue: x-projections over all timesteps ----------------
        import os as _os
        _SKIP_PRO = _os.environ.get("K_SKIP_PRO", "0") == "1"
        _SKIP_EPI = _os.environ.get("K_SKIP_EPI", "0") == "1"
        _SKIP_LOOP = _os.environ.get("K_SKIP_LOOP", "0") == "1"
        if _SKIP_PRO:
            if with_gru:
                nc.vector.memset(gx_g[:], 0.0)
            if with_lstm:
                nc.vector.memset(gx_l[:], 0.0)
        for c in range(0 if _SKIP_PRO else TB // CH):
            xst = stg.tile([D, CH], BF16, tag="xst")
            nc.sync.dma_start(xst[:], xT[:, c * CH:(c + 1) * CH])
            if with_gru:
                for m in range(NG_G):
                    p = psx.tile([128, CH], F32, tag="px")
                    nc.tensor.matmul(p[:], gW0s[:, m * 128:(m + 1) * 128],
                                     xst[:], start=True, stop=True)
                    nc.vector.tensor_scalar(
                        gx_g[:, m, c * CH:(c + 1) * CH], p[:],
                        bxgs[:, m:m + 1], None, op0=OP.add)
            if with_lstm:
                for m in range(NG_L):
                    p = psx.tile([128, CH], F32, tag="px")
                    nc.tensor.matmul(p[:], lW0s[:, m * 128:(m + 1) * 128],
                                     xst[:], start=True, stop=True)
                    nc.vector.tensor_scalar(
                        gx_l[:, m, c * CH:(c + 1) * CH], p[:],
                        bxls[:, m:m + 1], None, op0=OP.add)

        # ---- recurrent loop --------------------------------------------
        def k2(w, k, m):
            """[128,128] lhsT slice: K-tile k, M-tile m of a [256, Mtot] weight."""
            mt = w.shape[1] // 2
            return w[:, k * mt + m * 128: k * mt + (m + 1) * 128]

        assert not fp8, "fp8 path disabled (accuracy)"

        def gru_math(p_zr, p_xh, p_hh, bias_zr, bias_xh, bias_rh, h, m_t, o_dst,
                     xh_from_psum=False):
            """p_*: PSUM APs; bias_* None -> skip."""
            if bias_zr is None:
                zr_in = p_zr
            else:
                szr = tmp.tile([128, 4 * b], F32, tag="szr")
                nc.vector.tensor_add(szr[:], p_zr, bias_zr)
                zr_in = szr[:]
            zr = tmp.tile([128, 4 * b], BF16, tag="zr")
            nc.scalar.activation(zr[:], zr_in, AF.Sigmoid)
            w_ = tmp.tile([128, 2 * b], F32, tag="w_")
            if bias_rh is None:
                nc.vector.tensor_mul(w_[:], zr[:, 2 * b:4 * b], p_hh)
            else:
                v = tmp.tile([128, 2 * b], F32, tag="v")
                nc.vector.tensor_add(v[:], p_hh, bias_rh)
                nc.vector.tensor_mul(w_[:], zr[:, 2 * b:4 * b], v[:])
            sh = tmp.tile([128, 2 * b], F32, tag="sh")
            if bias_xh is None:
                nc.vector.tensor_add(sh[:], w_[:], p_xh)
            else:
                sh2 = tmp.tile([128, 2 * b], F32, tag="sh2")
                nc.vector.tensor_add(sh2[:], p_xh, bias_xh)
                nc.vector.tensor_add(sh[:], w_[:], sh2[:])
            hh = tmp.tile([128, 2 * b], BF16, tag="hh")
            nc.scalar.activation(hh[:], sh[:], AF.Tanh)
            d = tmp.tile([128, 2 * b], BF16, tag="d")
            nc.vector.tensor_sub(d[:], h[:], hh[:])
            e = tmp.tile([128, 2 * b], BF16, tag="e")
            nc.vector.tensor_mul(e[:], zr[:, 0:2 * b], d[:])
            cand = tmp.tile([128, 2 * b], BF16, tag="cand")
            nc.vector.tensor_add(cand[:], hh[:], e[:])
            nc.vector.copy_predicated(h[:], m_t, cand[:])
            if o_dst is not None:
                nc.gpsimd.tensor_copy(o_dst, h[:])

        def lstm_math(p_g_full, gx_or_bias, h, c_, m_t, o_dst):
            if gx_or_bias is None:
                g = p_g_full
            else:
                gt = tmp.tile([128, NG_L * b], F32, tag="g")
                nc.vector.tensor_add(gt[:], p_g_full, gx_or_bias)
                g = gt[:]
            ifo = tmp.tile([128, 6 * b], BF16, tag="ifo")
            nc.scalar.activation(ifo[:], g[:, 0:6 * b], AF.Sigmoid)
            ct = tmp.tile([128, 2 * b], BF16, tag="ct")
            nc.scalar.activation(ct[:], g[:, 6 * b:8 * b], AF.Tanh)
            a1 = tmp.tile([128, 2 * b], F32, tag="a1")
            nc.vector.tensor_mul(a1[:], ifo[:, 2 * b:4 * b], c_[:])
            a2 = tmp.tile([128, 2 * b], F32, tag="a2")
            nc.vector.tensor_mul(a2[:], ifo[:, 0:2 * b], ct[:])
            cn = tmp.tile([128, 2 * b], F32, tag="cn")
            nc.vector.tensor_add(cn[:], a1[:], a2[:])
            nc.vector.copy_predicated(c_[:], m_t, cn[:])
            tch = tmp.tile([128, 2 * b], BF16, tag="tch")
            nc.scalar.activation(tch[:], c_[:], AF.Tanh)
            hc = tmp.tile([128, 2 * b], BF16, tag="hc")
            nc.vector.tensor_mul(hc[:], ifo[:, 4 * b:6 * b], tch[:])
            nc.vector.copy_predicated(h[:], m_t, hc[:])
            if o_dst is not None:
                nc.gpsimd.tensor_copy(o_dst, h[:])

        ORDER = os.environ.get("K_ORDER", "V1a")

        def step(off_b, off_2b):
            m_t = mk_sb[:, ds(off_2b, 2 * b)]
            if with_gru:
                pG0 = psr.tile([128, NG_G * b], F32, tag="pG0")
                # V0 regions: zr [0,4b) | xh [4b,6b) | hh [6b,8b)
                # V1 adds:    zrU [8b,12b)
                pG1 = psr.tile([128, 12 * b], F32, tag="pG1")
            if with_lstm:
                pL0 = psr.tile([128, NG_L * b], F32, tag="pL0")
                # V0: g [0,8b) accumulates U+W; V1: W [0,8b) | U [8b,16b)
                pL1 = psr.tile([128, 16 * b], F32, tag="pL1")
            uzr = ul1 = None

            # ---- U-side L1 products (need h1(t-1)) ----
            if with_gru:
                for m in range(4, 6):       # hh region: U only, complete group
                    for k in range(2):
                        nc.tensor.matmul(pG1[:, (m + 2) * b:(m + 3) * b],
                                         k2(gU1s, k, m), hG1[:, k * b:(k + 1) * b],
                                         start=(k == 0), stop=(k == 1))
                if ORDER == "V0":
                    for m in range(4):      # zr: U part opens the group
                        for k in range(2):
                            nc.tensor.matmul(pG1[:, m * b:(m + 1) * b],
                                             k2(gU1s, k, m), hG1[:, k * b:(k + 1) * b],
                                             start=(k == 0), stop=False)
                else:
                    for m in range(4):      # zrU: separate complete groups
                        for k in range(2):
                            nc.tensor.matmul(pG1[:, (8 + m) * b:(9 + m) * b],
                                             k2(gU1s, k, m), hG1[:, k * b:(k + 1) * b],
                                             start=(k == 0), stop=(k == 1))
            if with_lstm:
                off_u = 0 if ORDER == "V0" else 8
                for m in range(NG_L):
                    for k in range(2):
                        nc.tensor.matmul(pL1[:, (off_u + m) * b:(off_u + m + 1) * b],
                                         k2(lU1s, k, m), hL1[:, k * b:(k + 1) * b],
                                         start=(k == 0),
                                         stop=(k == 1 and ORDER != "V0"))
            if ORDER != "V0":
                act_evac = ORDER == "V1a"
                if with_gru:
                    uzr = tmp.tile([128, 4 * b], F32, tag="uzr")
                    if act_evac:
                        nc.scalar.copy(uzr[:], pG1[:, 8 * b:12 * b])
                    else:
                        nc.vector.tensor_copy(uzr[:], pG1[:, 8 * b:12 * b])
                if with_lstm:
                    ul1 = tmp.tile([128, 8 * b], F32, tag="ul1")
                    if act_evac:
                        nc.scalar.copy(ul1[:], pL1[:, 8 * b:16 * b])
                    else:
                        nc.vector.tensor_copy(ul1[:], pL1[:, 8 * b:16 * b])

            # ---- layer-0 recurrent matmuls ----
            if with_gru:
                for m in range(NG_G):
                    for k in range(2):
                        nc.tensor.matmul(pG0[:, m * b:(m + 1) * b],
                                         k2(gU0s, k, m), hG0[:, k * b:(k + 1) * b],
                                         start=(k == 0), stop=(k == 1))
            if with_lstm:
                for m in range(NG_L):
                    for k in range(2):
                        nc.tensor.matmul(pL0[:, m * b:(m + 1) * b],
                                         k2(lU0s, k, m), hL0[:, k * b:(k + 1) * b],
                                         start=(k == 0), stop=(k == 1))

            # ---- layer-0 gate math ----
            if with_gru:
                gru_math(pG0[:, 0:4 * b], gx_g[:, 4:6, ds(off_b, b)],
                         pG0[:, 4 * b:6 * b],
                         gx_g[:, 0:4, ds(off_b, b)], None,
                         None if zero_bias else brh0s[:],
                         hG0, m_t, None)
            if with_lstm:
                lstm_math(pL0[:], gx_l[:, :, ds(off_b, b)], hL0, cL0, m_t, None)

            # ---- W-side L1 products (need h0(t)) ----
            if with_gru:
                for m in range(4):
                    for k in range(2):
                        nc.tensor.matmul(pG1[:, m * b:(m + 1) * b],
                                         k2(gW1s, k, m), hG0[:, k * b:(k + 1) * b],
                                         start=(ORDER != "V0" and k == 0),
                                         stop=(k == 1))
                for m in range(4, 6):       # xh region: W only
                    for k in range(2):
                        nc.tensor.matmul(pG1[:, m * b:(m + 1) * b],
                                         k2(gW1s, k, m), hG0[:, k * b:(k + 1) * b],
                                         start=(k == 0), stop=(k == 1))
            if with_lstm:
                for m in range(NG_L):
                    for k in range(2):
                        nc.tensor.matmul(pL1[:, m * b:(m + 1) * b],
                                         k2(lW1s, k, m), hL0[:, k * b:(k + 1) * b],
                                         start=(ORDER != "V0" and k == 0),
                                         stop=(k == 1))

            # ---- layer-1 gate math ----
            if with_gru:
                if ORDER == "V0":
                    p_zr = pG1[:, 0:4 * b]
                else:
                    szrl1 = tmp.tile([128, 4 * b], F32, tag="szrl1")
                    nc.vector.tensor_add(szrl1[:], uzr[:], pG1[:, 0:4 * b])
                    p_zr = szrl1[:]
                gru_math(p_zr, pG1[:, 4 * b:6 * b], pG1[:, 6 * b:8 * b],
                         None if zero_bias else bzr1s[:],
                         None if zero_bias else bxh1s[:],
                         None if zero_bias else brh1s[:],
                         hG1, m_t, o1[:, :, ds(off_b, b)])
            if with_lstm:
                if ORDER == "V0":
                    p_g = pL1[:, 0:8 * b]
                else:
                    gl1 = tmp.tile([128, 8 * b], F32, tag="gl1")
                    nc.vector.tensor_add(gl1[:], ul1[:], pL1[:, 0:8 * b])
                    p_g = gl1[:]
                lstm_math(p_g, None if zero_bias else bl1s[:], hL1, cL1, m_t,
                          o2[:, :, ds(off_b, b)])


        # ---- pipelined GRU-only body: interleave G1-math(t-1) x G0-math(t)
        def gru_mm_l0(pG0, h0):
            for m in range(NG_G):
                for k in range(2):
                    nc.tensor.matmul(pG0[:, m * b:(m + 1) * b],
                                     k2(gU0s, k, m), h0[:, k * b:(k + 1) * b],
                                     start=(k == 0), stop=(k == 1))

        def gru_mm_u(pG1, h1):
            for m in range(4, 6):
                for k in range(2):
                    nc.tensor.matmul(pG1[:, (m + 2) * b:(m + 3) * b],
                                     k2(gU1s, k, m), h1[:, k * b:(k + 1) * b],
                                     start=(k == 0), stop=(k == 1))
            for m in range(4):
                for k in range(2):
                    nc.tensor.matmul(pG1[:, (8 + m) * b:(9 + m) * b],
                                     k2(gU1s, k, m), h1[:, k * b:(k + 1) * b],
                                     start=(k == 0), stop=(k == 1))

        def gru_mm_w(pG1, h0):
            for m in range(4):
                for k in range(2):
                    nc.tensor.matmul(pG1[:, m * b:(m + 1) * b],
                                     k2(gW1s, k, m), h0[:, k * b:(k + 1) * b],
                                     start=(k == 0), stop=(k == 1))
            for m in range(4, 6):
                for k in range(2):
                    nc.tensor.matmul(pG1[:, m * b:(m + 1) * b],
                                     k2(gW1s, k, m), h0[:, k * b:(k + 1) * b],
                                     start=(k == 0), stop=(k == 1))

        class GChain:
            """One gate-math chain (either layer), emitted in stages."""
            def __init__(self, p_zr, p_xh, p_hh, xh_sbuf, h, m_t, o_dst):
                self.p_zr, self.p_xh, self.p_hh = p_zr, p_xh, p_hh
                self.xh_sbuf = xh_sbuf
                self.h, self.m_t, self.o_dst = h, m_t, o_dst

            def s_sigma(self):
                self.zr = tmp.tile([128, 4 * b], BF16, tag="zr")
                nc.scalar.activation(self.zr[:], self.p_zr, AF.Sigmoid)

            def s_wsh(self):
                self.sh = tmp.tile([128, 2 * b], F32, tag="sh")
                w_ = tmp.tile([128, 2 * b], F32, tag="w_")
                nc.vector.tensor_mul(w_[:], self.zr[:, 2 * b:4 * b], self.p_hh)
                nc.vector.tensor_add(self.sh[:], w_[:], self.p_xh)

            def s_tanh(self):
                self.hh = tmp.tile([128, 2 * b], BF16, tag="hh")
                nc.scalar.activation(self.hh[:], self.sh[:], AF.Tanh)

            def s_update(self):
                d = tmp.tile([128, 2 * b], BF16, tag="d")
                nc.vector.tensor_sub(d[:], self.h[:], self.hh[:])
                e = tmp.tile([128, 2 * b], BF16, tag="e")
                nc.vector.tensor_mul(e[:], self.zr[:, 0:2 * b], d[:])
                cand = tmp.tile([128, 2 * b], BF16, tag="cand")
                nc.vector.tensor_add(cand[:], self.hh[:], e[:])
                nc.vector.copy_predicated(self.h[:], self.m_t, cand[:])
                if self.o_dst is not None:
                    nc.gpsimd.tensor_copy(self.o_dst, self.h[:])

        def gru_pipelined_body(i):
            prev = None          # G1 chain of previous j
            prev_pG1 = None
            prev_uzr = None
            for j in range(unroll):
                off_b = i * (unroll * b) + j * b
                off_2b = i * (unroll * 2 * b) + j * 2 * b
                m_t = mk_sb[:, ds(off_2b, 2 * b)]
                pG0 = psr.tile([128, NG_G * b], F32, tag="pG0")
                pG1 = psr.tile([128, 12 * b], F32, tag="pG1")
                gru_mm_l0(pG0, hG0)
                if prev is not None:
                    gru_mm_w(prev_pG1, hG0)
                gru_mm_u(pG1, hG1)
                # assemble G0 chain for this step
                szr = tmp.tile([128, 4 * b], F32, tag="szr")
                nc.vector.tensor_add(szr[:], pG0[:, 0:4 * b],
                                     gx_g[:, 0:4, ds(off_b, b)])
                g0 = GChain(szr[:], gx_g[:, 4:6, ds(off_b, b)],
                            pG0[:, 4 * b:6 * b], True, hG0, m_t, None)
                seqmode = os.environ.get("K_GRUPIPE", "1") == "2"
                if prev is not None:
                    szrl1 = tmp.tile([128, 4 * b], F32, tag="szrl1")
                    nc.vector.tensor_add(szrl1[:], prev_uzr[:],
                                         prev_pG1[:, 0:4 * b])
                    prev.p_zr = szrl1[:]
                    if seqmode:
                        prev.s_sigma(); prev.s_wsh(); prev.s_tanh(); prev.s_update()
                    else:
                        prev.s_sigma()
                if not seqmode and prev is not None:
                    g0.s_sigma()
                    prev.s_wsh()
                    g0.s_wsh()
                    prev.s_tanh()
                    g0.s_tanh()
                    prev.s_update()
                    g0.s_update()
                else:
                    g0.s_sigma(); g0.s_wsh(); g0.s_tanh(); g0.s_update()
                uzr = tmp.tile([128, 4 * b], F32, tag="uzr")
                nc.vector.tensor_copy(uzr[:], pG1[:, 8 * b:12 * b])
                prev = GChain(None, pG1[:, 4 * b:6 * b], pG1[:, 6 * b:8 * b],
                              False, hG1, m_t, o1[:, :, ds(off_b, b)])
                prev_pG1, prev_uzr = pG1, uzr
            # tail: finish G1 chain of the last step in this body
            gru_mm_w(prev_pG1, hG0)
            szrl1 = tmp.tile([128, 4 * b], F32, tag="szrl1")
            nc.vector.tensor_add(szrl1[:], prev_uzr[:], prev_pG1[:, 0:4 * b])
            prev.p_zr = szrl1[:]
            prev.s_sigma(); prev.s_wsh(); prev.s_tanh(); prev.s_update()

        use_pipe = (with_gru and not with_lstm and zero_bias
                    and os.environ.get("K_GRUPIPE", "0") in ("1", "2"))

        n_outer = T // unroll
        hint = tuple()
        if os.environ.get("K_HINT", "0") == "1":
            hint = (mybir.EngineType.PE, mybir.EngineType.DVE,
                    mybir.EngineType.Activation)
        stag = os.environ.get("K_STAG", "0") == "1"
        if not _SKIP_LOOP:
            if time_mult == 1:
                with tc.For_i(0, n_outer, 1, hint_engines=hint,
                              staggered_reset=stag) as i:
                    if use_pipe:
                        gru_pipelined_body(i)
                    else:
                        for j in range(unroll):
                            step(i * (unroll * b) + j * b,
                                 i * (unroll * 2 * b) + j * 2 * b)
            else:
                with tc.For_i(0, time_mult, 1) as _rep:
                    with tc.For_i(0, n_outer, 1, hint_engines=hint,
                                  staggered_reset=stag) as i:
                        if use_pipe:
                            gru_pipelined_body(i)
                        else:
                            for j in range(unroll):
                                step(i * (unroll * b) + j * b,
                                     i * (unroll * 2 * b) + j * 2 * b)
        else:
            if with_gru:
                nc.vector.memset(o1[:], 0.0)
            if with_lstm:
                nc.vector.memset(o2[:], 0.0)

        # ---- epilogue: dense head --------------------------------------
        if _SKIP_EPI:
            y_sb0 = tmp.tile([1, CH], F32, tag="ych")
            nc.vector.memset(y_sb0[:], 0.0)
            nc.sync.dma_start(y_ext[:, 0:CH], y_sb0[:])
        for c in range(0 if _SKIP_EPI else TB // CH):
            py = psx.tile([1, CH], F32, tag="py")
            srcs = []
            if with_gru:
                srcs += [o1[:, 0, c * CH:(c + 1) * CH], o1[:, 1, c * CH:(c + 1) * CH]]
            if with_lstm:
                srcs += [o2[:, 0, c * CH:(c + 1) * CH], o2[:, 1, c * CH:(c + 1) * CH]]
            for ki, s in enumerate(srcs):
                nc.tensor.matmul(py[:], wo_sb[:, ki:ki + 1], s,
                                 start=(ki == 0), stop=(ki == len(srcs) - 1))
            ych = tmp.tile([1, CH], F32, tag="ych")
            nc.vector.tensor_copy(ych[:], py[:])
            nc.sync.dma_start(y_ext[:, c * CH:(c + 1) * CH], ych[:])

    nc.finalize()
    return nc


def build_nc2(b, model, zero_bias=False, chunk=None):
    """v2: chunked software pipeline. L1 runs one chunk (C steps) behind L0;
    the L1 input projections W1 @ h0 are computed as bulk matmuls (free dim
    C*b=128) once per chunk instead of per-step, cutting the per-step PE
    instruction count from 48 to 32+2 (LSTM) / 36 to 24+1.5 (GRU). The PE
    pair cost is ~flat in free-dim (N=16 ~92ns vs N=128 ~110ns), so bulk
    W1 work is nearly free.

    model: "gru" | "lstm". Single-model builds only (p2 dispatch).
    """
    C = chunk or UNROLL
    nc = bacc.Bacc()
    TB = T * b
    with_gru = model == "gru"
    NG = NG_G if with_gru else NG_L
    n_chunks = T // C
    assert n_chunks % 2 == 0 and n_chunks >= 4

    xT = nc.declare_dram_parameter("xT", [D, TB], BF16, isOutput=False)
    mk = nc.declare_dram_parameter("mask", [128, T * 2 * b], U8, isOutput=False)
    y_ext = nc.declare_dram_parameter("y", [1, TB], F32, isOutput=True)

    def param(name, shape, dt=BF16):
        return nc.declare_dram_parameter(name, shape, dt, isOutput=False)

    if with_gru:
        W0 = param("gW0", [D, 768])
        U0 = param("gU0", [128, 2 * 768])
        W1 = param("gW1", [128, 2 * 768])
        U1 = param("gU1", [128, 2 * 768])
        bx = param("bxg", [128, NG], F32)
        bzr1 = param("bzr1", [128, 4 * b], F32)
        bxh1 = param("bxh1", [128, 2 * b], F32)
        brh1 = param("brh1", [128, 2 * b], F32)
        brh0 = param("brh0", [128, 2 * b], F32)
    else:
        W0 = param("lW0", [D, 1024])
        U0 = param("lU0", [128, 2 * 1024])
        W1 = param("lW1", [128, 2 * 1024])
        U1 = param("lU1", [128, 2 * 1024])
        bx = param("bxl", [128, NG], F32)
        bl1 = param("bl1", [128, NG * b], F32)
    wo = param("wo", [128, 2])

    trace_sim = os.environ.get("K_TRACE", "0") == "1"
    with tile.TileContext(nc, trace_sim=trace_sim) as tc, ExitStack() as ctx:
        pool = ctx.enter_context(tc.tile_pool(name="main", bufs=1))
        stg = ctx.enter_context(tc.tile_pool(name="stg", bufs=2))
        tmp = ctx.enter_context(tc.tile_pool(name="tmp", bufs=2))
        psx = ctx.enter_context(tc.tile_pool(name="psx", bufs=1, space="PSUM"))
        psr = ctx.enter_context(tc.tile_pool(name="psr", bufs=2, space="PSUM"))
        psw = ctx.enter_context(tc.tile_pool(name="psw", bufs=1, space="PSUM"))

        mk_sb = pool.tile([128, T * 2 * b], U8)
        nc.sync.dma_start(mk_sb[:], mk[:])
        wo_sb = pool.tile([128, 2], BF16)
        nc.sync.dma_start(wo_sb[:], wo[:])

        def load(p, shape, dt=BF16):
            t_ = pool.tile(shape, dt, tag=f"w_{p.name}")
            nc.sync.dma_start(t_[:], p[:])
            return t_

        W0s = load(W0, [D, NG * 128])
        U0s = load(U0, [128, 2 * NG * 128])
        W1s = load(W1, [128, 2 * NG * 128])
        U1s = load(U1, [128, 2 * NG * 128])
        bxs = load(bx, [128, NG], F32)
        if with_gru:
            bzr1s = load(bzr1, [128, 4 * b], F32)
            bxh1s = load(bxh1, [128, 2 * b], F32)
            brh1s = load(brh1, [128, 2 * b], F32)
            brh0s = load(brh0, [128, 2 * b], F32)
        else:
            bl1s = load(bl1, [128, NG * b], F32)

        gx = pool.tile([128, NG, TB], BF16)       # L0 x-proj, all timesteps
        oh = pool.tile([128, 2, TB], BF16)        # L1 output history (head in)
        h0 = pool.tile([128, 2 * b], BF16)
        h1 = pool.tile([128, 2 * b], BF16)
        nc.vector.memset(h0[:], 0.0)
        nc.vector.memset(h1[:], 0.0)
        if not with_gru:
            c0 = pool.tile([128, 2 * b], F32)
            c1 = pool.tile([128, 2 * b], F32)
            nc.vector.memset(c0[:], 0.0)
            nc.vector.memset(c1[:], 0.0)
        hist0 = pool.tile([128, 2, C * b], BF16, tag="hist0")
        hist1 = pool.tile([128, 2, C * b], BF16, tag="hist1")
        w1ev0 = pool.tile([128, NG, C * b], BF16, tag="w1ev0")
        w1ev1 = pool.tile([128, NG, C * b], BF16, tag="w1ev1")
        hist = [hist0, hist1]
        w1ev = [w1ev0, w1ev1]

        # ---- prologue: x-projections over all timesteps ----------------
        for cc in range(TB // CH):
            xst = stg.tile([D, CH], BF16, tag="xst")
            nc.sync.dma_start(xst[:], xT[:, cc * CH:(cc + 1) * CH])
            for m in range(NG):
                p = psx.tile([128, CH], F32, tag="px")
                nc.tensor.matmul(p[:], W0s[:, m * 128:(m + 1) * 128],
                                 xst[:], start=True, stop=True)
                nc.vector.tensor_scalar(
                    gx[:, m, cc * CH:(cc + 1) * CH], p[:],
                    bxs[:, m:m + 1], None, op0=OP.add)

        def k2(w, k, m):
            mt = w.shape[1] // 2
            return w[:, k * mt + m * 128: k * mt + (m + 1) * 128]

        def mm_rec(ptile, Ws, hsrc):
            for m in range(NG):
                for k in range(2):
                    nc.tensor.matmul(ptile[:, m * b:(m + 1) * b],
                                     k2(Ws, k, m), hsrc[:, k * b:(k + 1) * b],
                                     start=(k == 0), stop=(k == 1))

        def gru_cell(szr, xh_src, p_hh, bias_rh, h, m_t, hist_dst):
            """szr: [128,4b] f32 pre-activation for z,r (already summed).
            xh_src: [128,(2),b] additive candidate input (x/W1 side).
            p_hh: [128,2b] recurrent candidate part (U side, pre-bias)."""
            zr = tmp.tile([128, 4 * b], BF16, tag="zr")
            nc.scalar.activation(zr[:], szr, AF.Sigmoid)
            w_ = tmp.tile([128, 2 * b], F32, tag="w_")
            if bias_rh is None:
                nc.vector.tensor_mul(w_[:], zr[:, 2 * b:4 * b], p_hh)
            else:
                v = tmp.tile([128, 2 * b], F32, tag="v")
                nc.vector.tensor_add(v[:], p_hh, bias_rh)
                nc.vector.tensor_mul(w_[:], zr[:, 2 * b:4 * b], v[:])
            sh = tmp.tile([128, 2 * b], F32, tag="sh")
            nc.vector.tensor_add(sh[:], w_[:], xh_src)
            hh = tmp.tile([128, 2 * b], BF16, tag="hh")
            nc.scalar.activation(hh[:], sh[:], AF.Tanh)
            d = tmp.tile([128, 2 * b], BF16, tag="d")
            nc.vector.tensor_sub(d[:], h[:], hh[:])
            e = tmp.tile([128, 2 * b], BF16, tag="e")
            nc.vector.tensor_mul(e[:], zr[:, 0:2 * b], d[:])
            cand = tmp.tile([128, 2 * b], BF16, tag="cand")
            nc.vector.tensor_add(cand[:], hh[:], e[:])
            nc.vector.copy_predicated(h[:], m_t, cand[:])
            if hist_dst is not None:
                nc.gpsimd.tensor_copy(hist_dst, h[:])

        def lstm_cell(g, h, c_, m_t, hist_dst):
            ifo = tmp.tile([128, 6 * b], BF16, tag="ifo")
            nc.scalar.activation(ifo[:], g[:, 0:6 * b], AF.Sigmoid)
            ct = tmp.tile([128, 2 * b], BF16, tag="ct")
            nc.scalar.activation(ct[:], g[:, 6 * b:8 * b], AF.Tanh)
            a1 = tmp.tile([128, 2 * b], F32, tag="a1")
            nc.vector.tensor_mul(a1[:], ifo[:, 2 * b:4 * b], c_[:])
            a2 = tmp.tile([128, 2 * b], F32, tag="a2")
            nc.vector.tensor_mul(a2[:], ifo[:, 0:2 * b], ct[:])
            cn = tmp.tile([128, 2 * b], F32, tag="cn")
            nc.vector.tensor_add(cn[:], a1[:], a2[:])
            nc.vector.copy_predicated(c_[:], m_t, cn[:])
            tch = tmp.tile([128, 2 * b], BF16, tag="tch")
            nc.scalar.activation(tch[:], c_[:], AF.Tanh)
            hc = tmp.tile([128, 2 * b], BF16, tag="hc")
            nc.vector.tensor_mul(hc[:], ifo[:, 4 * b:6 * b], tch[:])
            nc.vector.copy_predicated(h[:], m_t, hc[:])
            if hist_dst is not None:
                nc.gpsimd.tensor_copy(hist_dst, h[:])

        def a_step(off_b, off_2b, par, j):
            """L0 step t; h0 recurrence + history write for bulk W1."""
            m_t = mk_sb[:, ds(off_2b, 2 * b)]
            pA = psr.tile([128, NG * b], F32, tag="pA")
            mm_rec(pA, U0s, h0)
            hd = hist[par][:, :, j * b:(j + 1) * b]
            if with_gru:
                szr = tmp.tile([128, 4 * b], F32, tag="szrA")
                nc.vector.tensor_add(szr[:], pA[:, 0:4 * b],
                                     gx[:, 0:4, ds(off_b, b)])
                gru_cell(szr[:], gx[:, 4:6, ds(off_b, b)], pA[:, 4 * b:6 * b],
                         None if zero_bias else brh0s[:], h0, m_t, hd)
            else:
                gt = tmp.tile([128, NG * b], F32, tag="gtA")
                nc.vector.tensor_add(gt[:], pA[:], gx[:, :, ds(off_b, b)])
                lstm_cell(gt, h0, c0, m_t, hd)

        def c_step(off_b, off_2b, par, j):
            """L1 step t; reads bulk W1 products of its chunk (parity par)."""
            m_t = mk_sb[:, ds(off_2b, 2 * b)]
            pC = psr.tile([128, NG * b], F32, tag="pC")
            mm_rec(pC, U1s, h1)
            od = oh[:, :, ds(off_b, b)]
            wj = w1ev[par][:, :, j * b:(j + 1) * b]
            if with_gru:
                szr = tmp.tile([128, 4 * b], F32, tag="szrC")
                nc.vector.tensor_add(szr[:], pC[:, 0:4 * b],
                                     w1ev[par][:, 0:4, j * b:(j + 1) * b])
                if not zero_bias:
                    szr2 = tmp.tile([128, 4 * b], F32, tag="szrC2")
                    nc.vector.tensor_add(szr2[:], szr[:], bzr1s[:])
                    szr = szr2
                xh = w1ev[par][:, 4:6, j * b:(j + 1) * b]
                if not zero_bias:
                    xh2 = tmp.tile([128, 2 * b], F32, tag="xhC")
                    nc.vector.tensor_add(xh2[:], xh, bxh1s[:])
                    xh = xh2[:]
                gru_cell(szr[:], xh, pC[:, 4 * b:6 * b],
                         None if zero_bias else brh1s[:], h1, m_t, od)
            else:
                gt = tmp.tile([128, NG * b], F32, tag="gtC")
                nc.vector.tensor_add(gt[:], pC[:], wj)
                if not zero_bias:
                    gt2 = tmp.tile([128, NG * b], F32, tag="gtC2")
                    nc.vector.tensor_add(gt2[:], gt[:], bl1s[:])
                    gt = gt2
                lstm_cell(gt, h1, c1, m_t, od)

        def bulk_w1(par):
            pW = psw.tile([128, NG, C * b], F32, tag="pW")
            for m in range(NG):
                for k in range(2):
                    nc.tensor.matmul(pW[:, m, :], k2(W1s, k, m),
                                     hist[par][:, k, :],
                                     start=(k == 0), stop=(k == 1))
            nc.scalar.copy(w1ev[par][:], pW[:])

        def half(base_b, base_2b, k_par):
            """A-steps of chunk k (parity k_par) + C-steps of chunk k-1,
            then bulk W1 of chunk k. base_* = element offsets of chunk k."""
            for j in range(C):
                a_step(base_b + j * b, base_2b + j * 2 * b, k_par, j)
                c_step(base_b - C * b + j * b, base_2b - 2 * C * b + j * 2 * b,
                       1 - k_par, j)
            bulk_w1(k_par)

        # ---- prologue chunk 0 ------------------------------------------
        for j in range(C):
            a_step(j * b, j * 2 * b, 0, j)
        bulk_w1(0)

        # ---- main loop: halves k=2i+1, k=2i+2 --------------------------
        n_outer = (n_chunks - 2) // 2
        with tc.For_i(0, n_outer, 1) as i:
            half(i * (2 * C * b) + C * b, i * (4 * C * b) + 2 * C * b, 1)
            half(i * (2 * C * b) + 2 * C * b, i * (4 * C * b) + 4 * C * b, 0)

        # ---- epilogue: chunk 63 + final C chunk ------------------------
        last = n_chunks - 1
        half(last * C * b, last * 2 * C * b, last % 2)
        for j in range(C):
            c_step(last * C * b + j * b, last * 2 * C * b + j * 2 * b,
                   last % 2, j)

        # ---- dense head ------------------------------------------------
        for cc in range(TB // CH):
            py = psx.tile([1, CH], F32, tag="py")
            srcs = [oh[:, 0, cc * CH:(cc + 1) * CH],
                    oh[:, 1, cc * CH:(cc + 1) * CH]]
            for ki, s in enumerate(srcs):
                nc.tensor.matmul(py[:], wo_sb[:, ki:ki + 1], s,
                                 start=(ki == 0), stop=(ki == len(srcs) - 1))
            ych = tmp.tile([1, CH], F32, tag="ych")
            nc.vector.tensor_copy(ych[:], py[:])
            nc.sync.dma_start(y_ext[:, cc * CH:(cc + 1) * CH], ych[:])

    nc.finalize()
    return nc


# ---------------------------------------------------------------------------
# Host-side data preparation
# ---------------------------------------------------------------------------

def _fold_k(w):
    """[256, M] -> [128, 2*M] with K-tile-major free layout."""
    m = w.shape[1]
    return np.ascontiguousarray(
        w.reshape(2, 128, m).transpose(1, 0, 2).reshape(128, 2 * m))


def _ptile(v):
    """[n*128] vector -> [128, n] per-partition tile layout."""
    n = v.shape[0] // 128
    return np.ascontiguousarray(v.reshape(n, 128).T)


def _bcast_b(v2, b):
    """[128, n] -> [128, n*b] replicated along batch."""
    return np.ascontiguousarray(
        np.repeat(v2[:, :, None], b, axis=2).reshape(128, -1))


_LSTM_PERM = np.r_[0:512, 768:1024, 512:768]   # i,f,c,o -> i,f,o,c


def prep_shared(inputs, b, with_gru=True, with_lstm=True, fp8=False):
    f32 = np.float32
    bf = ml_dtypes.bfloat16
    wdt = ml_dtypes.float8_e4m3 if fp8 else bf
    wsc = WSCALE if fp8 else 1.0
    out = {}
    if with_gru:
        bi0, br0 = inputs["g_bi0"].astype(f32), inputs["g_br0"].astype(f32)
        bx = np.concatenate([(bi0 + br0)[:512], bi0[512:]])
        out["gW0"] = np.ascontiguousarray(inputs["g_W0"]).astype(bf)
        out["gU0"] = (_fold_k(inputs["g_U0"].astype(f32)) * wsc).astype(wdt)
        out["gW1"] = (_fold_k(inputs["g_W1"].astype(f32)) * wsc).astype(wdt)
        out["gU1"] = (_fold_k(inputs["g_U1"].astype(f32)) * wsc).astype(wdt)
        out["bxg"] = _ptile(bx)
        bi1, br1 = inputs["g_bi1"].astype(f32), inputs["g_br1"].astype(f32)
        out["bzr1"] = _bcast_b(_ptile((bi1 + br1)[:512]), b)
        out["bxh1"] = _bcast_b(_ptile(bi1[512:]), b)
        out["brh1"] = _bcast_b(_ptile(br1[512:]), b)
        out["brh0"] = _bcast_b(_ptile(br0[512:]), b)
    if with_lstm:
        out["lW0"] = np.ascontiguousarray(
            inputs["l_W0"][:, _LSTM_PERM]).astype(bf)
        out["lU0"] = (_fold_k(inputs["l_U0"][:, _LSTM_PERM].astype(f32)) * wsc).astype(wdt)
        out["lW1"] = (_fold_k(inputs["l_W1"][:, _LSTM_PERM].astype(f32)) * wsc).astype(wdt)
        out["lU1"] = (_fold_k(inputs["l_U1"][:, _LSTM_PERM].astype(f32)) * wsc).astype(wdt)
        out["bxl"] = _ptile(inputs["l_b0"][_LSTM_PERM].astype(f32))
        out["bl1"] = _bcast_b(_ptile(inputs["l_b1"][_LSTM_PERM].astype(f32)), b)
    w = inputs["out_W"][:, 0].astype(f32)        # [512]
    cols = []
    if with_gru:
        cols += [w[0:128], w[128:256]]
    if with_lstm:
        cols += [w[256:384], w[384:512]]
    out["wo"] = np.stack(cols, axis=1).astype(bf)
    return out


def prep_core(inputs, idx, b):
    """Per-core x transpose + mask broadcast for batch indices idx."""
    bf = ml_dtypes.bfloat16
    x = inputs["x"][idx]                          # [b, T, D]
    xT = np.ascontiguousarray(x.transpose(2, 1, 0).reshape(D, T * b)).astype(bf)
    lens = np.asarray(inputs["lengths"])[idx]
    m = (np.arange(T)[:, None] < lens[None, :])   # [T, b]
    mk = np.broadcast_to(m[None, :, None, :], (128, T, 2, b))
    mk = np.ascontiguousarray(mk.reshape(128, T * 2 * b)).astype(np.uint8)
    return {"xT": xT, "mask": mk}


_NC_CACHE = {}


def make_exec(nc, dev_off, n_cores):
    """Compile nc into a jitted shard_map over devices[dev_off:dev_off+n_cores].

    Returns (dispatch, finish): dispatch(in_maps) launches asynchronously and
    returns a handle; finish(handle) blocks and returns per-core result dicts.
    """
    import jax
    from jax.sharding import Mesh, PartitionSpec
    from jax.experimental.shard_map import shard_map
    from concourse.bass2jax import (_bass_exec_p, install_neuronx_cc_hook,
                                    partition_id_tensor)

    install_neuronx_cc_hook()
    partition_name = nc.partition_id_tensor.name if nc.partition_id_tensor else None
    in_names, out_names, out_avals = [], [], []
    for alloc in nc.m.functions[0].allocations:
        if not isinstance(alloc, mybir.MemoryLocationSet):
            continue
        name = alloc.memorylocations[0].name
        if alloc.kind == "ExternalInput":
            if name != partition_name:
                in_names.append(name)
        elif alloc.kind == "ExternalOutput":
            out_names.append(name)
            out_avals.append(jax.core.ShapedArray(
                tuple(alloc.tensor_shape), mybir.dt.np(alloc.dtype)))
    n_params = len(in_names)
    all_in = list(in_names) + list(out_names)
    if partition_name is not None:
        all_in.append(partition_name)

    def _body(*args):
        operands = list(args)
        if partition_name is not None:
            operands.append(partition_id_tensor())
        return tuple(_bass_exec_p.bind(
            *operands, out_avals=tuple(out_avals), in_names=tuple(all_in),
            out_names=tuple(out_names), lowering_input_output_aliases=(),
            sim_require_finite=True, sim_require_nnan=True, nc=nc))

    devices = jax.devices()[dev_off:dev_off + n_cores]
    mesh = Mesh(np.asarray(devices), ("core",))
    nio = n_params + len(out_avals)
    sharded = jax.jit(shard_map(_body, mesh=mesh,
                                in_specs=(PartitionSpec("core"),) * nio,
                                out_specs=(PartitionSpec("core"),) * len(out_avals),
                                check_rep=False), keep_unused=True)

    def dispatch(in_maps):
        concat_in = [np.concatenate([np.asarray(in_maps[c][n])
                                     for c in range(n_cores)], axis=0)
                     for n in in_names]
        zeros = [np.zeros((n_cores * a.shape[0], *a.shape[1:]), a.dtype)
                 for a in out_avals]
        return sharded(*concat_in, *zeros)

    def finish(handle):
        import jax
        jax.block_until_ready(handle)
        return [{n: np.asarray(handle[i]).reshape(n_cores, *out_avals[i].shape)[c]
                 for i, n in enumerate(out_names)} for c in range(n_cores)]

    dispatch.sharded = sharded
    dispatch.in_names = in_names
    dispatch.out_avals = out_avals
    dispatch.mesh = mesh
    return dispatch, finish


MODE = os.environ.get("K_MODE", "p2")


def _all_bias_zero(inputs):
    return all(not np.any(np.asarray(inputs[k]))
               for k in ("g_bi0", "g_br0", "g_bi1", "g_br1", "l_b0", "l_b1"))


FP8_ON = os.environ.get("K_FP8", "0") == "1"


K_IMPL = os.environ.get("K_IMPL", "v2")


def _get_p2(zb):
    key = ("p2", zb, FP8_ON, K_IMPL)
    if key not in _NC_CACHE:
        bl = B // (NCORES // 2)      # 16 examples per core
        if K_IMPL == "v2":
            nc_g = build_nc2(bl, "gru", zero_bias=zb)
            nc_l = build_nc2(bl, "lstm", zero_bias=zb)
        else:
            nc_g = build_nc(bl, True, False, zero_bias=zb, fp8=FP8_ON)
            nc_l = build_nc(bl, False, True, zero_bias=zb, fp8=FP8_ON)
        dg, fg = make_exec(nc_g, 0, NCORES // 2)
        dl, fl = make_exec(nc_l, NCORES // 2, NCORES // 2)
        _NC_CACHE[key] = (nc_g, nc_l, dg, fg, dl, fl, bl)
    return _NC_CACHE[key]


def kernel(**inputs) -> np.ndarray:
    out_b = float(np.asarray(inputs["out_b"]).reshape(-1)[0])
    y = np.empty((B, T, 1), np.float32)
    if MODE == "p1":
        if "p1" not in _NC_CACHE:
            _NC_CACHE["p1"] = build_nc(B_LOC, True, True)
        nc = _NC_CACHE["p1"]
        shared = prep_shared(inputs, B_LOC, True, True)
        in_maps = []
        for c in range(NCORES):
            m = dict(shared)
            m.update(prep_core(inputs, slice(c * B_LOC, (c + 1) * B_LOC), B_LOC))
            in_maps.append(m)
        res = run_bass_kernel_spmd(nc, in_maps, core_ids=list(range(NCORES)))
        for c in range(NCORES):
            yc = res.results[c]["y"].reshape(T, B_LOC)
            y[c * B_LOC:(c + 1) * B_LOC, :, 0] = yc.T + out_b
        return y

    nc_g, nc_l, dg, fg, dl, fl, bl = _get_p2(_all_bias_zero(inputs))
    hc = NCORES // 2
    sh_g = prep_shared(inputs, bl, True, False, fp8=FP8_ON)
    sh_l = prep_shared(inputs, bl, False, True, fp8=FP8_ON)
    maps_g, maps_l = [], []
    for c in range(hc):
        core = prep_core(inputs, slice(c * bl, (c + 1) * bl), bl)
        mg = dict(sh_g); mg.update(core); maps_g.append(mg)
        ml = dict(sh_l); ml.update(core); maps_l.append(ml)
    hg = dg(maps_g)
    hl = dl(maps_l)
    res_g = fg(hg)
    res_l = fl(hl)
    for c in range(hc):
        yc = (res_g[c]["y"].astype(np.float32)
              + res_l[c]["y"].astype(np.float32)).reshape(T, bl)
        y[c * bl:(c + 1) * bl, :, 0] = yc.T + out_b
    return y



# revision 8
# speedup vs baseline: 1.6714x; 1.6714x over previous
"""Trainium2 Bass kernel for AEDiscriminator: 2-layer GRU + 2-layer LSTM stacks
with length masking, concat + dense head.

Sharding (default mode "p2"): model-split x batch-split. The GRU stack runs on
cores 0-3 and the LSTM stack on cores 4-7, each group data-parallel over 16
examples/core. The two NEFFs are dispatched asynchronously on disjoint device
subsets. No inter-core communication: the dense head decomposes as
o1 @ W[:256] + o2 @ W[256:], summed on the host. Mode "p1" (K_MODE=p1) is a
single-graph fallback: all four layers on every core, 8 examples/core.

Per-core layout: features/gates on SBUF partitions (128-row tiles), batch on
the free dimension. Recurrent matmuls run weights-stationary (bf16):
out[gates, batch] = U[k, gates].T @ h[k, batch]. Layer-0 input projections
(x @ W + b) are precomputed for all timesteps as large matmuls in a prologue;
the T=512 recurrent loop is a hardware loop (For_i) with an 8-step unrolled
body and register-indexed access patterns; masking is a single copy_predicated
per state tensor. Layer-1 U-products are computed into separate PSUM regions
early (they need only h1(t-1)) so the PE never queue-stalls behind W-products
that wait on h0(t); ScalarE evacuates them. Per-step time is within ~15% of
the PE weight-load floor (~107 ns LDWEIGHTS per 128x128 tile; the toolchain's
walrus has --enable-ldw-opt=false, so fast-weight-load is unavailable).
"""

import os
from contextlib import ExitStack

import numpy as np
import ml_dtypes

import concourse.bass as bass
import concourse.tile as tile
from concourse import bacc, mybir
from concourse.bass_utils import run_bass_kernel_spmd
from concourse import bass_utils as _bu

if os.environ.get("K_LDWOPT", "0") == "1" and not getattr(_bu, "_ldw_patched", False):
    _orig_run_command = _bu.run_command

    def _patched_run_command(cmd, *a, **kw):
        cmd = [c.replace("--enable-ldw-opt=false", "--enable-ldw-opt=true")
               if isinstance(c, str) else c for c in cmd]
        return _orig_run_command(cmd, *a, **kw)

    _bu.run_command = _patched_run_command
    _bu._ldw_patched = True

BF16 = mybir.dt.bfloat16
FP8 = mybir.dt.float8e4
WSCALE = 16.0
F32 = mybir.dt.float32
U8 = mybir.dt.uint8
AF = mybir.ActivationFunctionType
OP = mybir.AluOpType
ds = bass.ds

B, T, D, H = 64, 512, 96, 256
NCORES = 8
B_LOC = B // NCORES          # 8 examples per core
UNROLL = int(os.environ.get("K_UNROLL", "8"))
NG_G, NG_L = 6, 8            # gate tiles of 128: GRU 768, LSTM 1024
CH = 512                     # free-dim chunk for bulk matmuls


def build_nc(b=B_LOC, with_gru=True, with_lstm=True, unroll=UNROLL,
             zero_bias=False, time_mult=1, fp8=False):
    nc = bacc.Bacc()
    TB = T * b

    xT = nc.declare_dram_parameter("xT", [D, TB], BF16, isOutput=False)
    mk = nc.declare_dram_parameter("mask", [128, T * 2 * b], U8, isOutput=False)
    y_ext = nc.declare_dram_parameter("y", [1, TB], F32, isOutput=True)

    def param(name, shape, dt=BF16):
        return nc.declare_dram_parameter(name, shape, dt, isOutput=False)

    WDT = FP8 if fp8 else BF16
    if with_gru:
        gW0 = param("gW0", [D, 768])
        gU0 = param("gU0", [128, 2 * 768], WDT)
        gW1 = param("gW1", [128, 2 * 768], WDT)
        gU1 = param("gU1", [128, 2 * 768], WDT)
        bxg = param("bxg", [128, NG_G], F32)        # x-proj bias per m-tile
        bzr1 = param("bzr1", [128, 4 * b], F32)     # L1 (bi+br) for z,r
        bxh1 = param("bxh1", [128, 2 * b], F32)     # L1 bi_h
        brh1 = param("brh1", [128, 2 * b], F32)     # L1 br_h
        brh0 = param("brh0", [128, 2 * b], F32)     # L0 br_h
    if with_lstm:
        lW0 = param("lW0", [D, 1024])
        lU0 = param("lU0", [128, 2 * 1024], WDT)
        lW1 = param("lW1", [128, 2 * 1024], WDT)
        lU1 = param("lU1", [128, 2 * 1024], WDT)
        bxl = param("bxl", [128, NG_L], F32)        # x-proj bias per m-tile
        bl1 = param("bl1", [128, NG_L * b], F32)    # L1 bias, broadcast over b
    n_head_k = (2 if with_gru else 0) + (2 if with_lstm else 0)
    wo = param("wo", [128, n_head_k])

    trace_sim = os.environ.get("K_TRACE", "0") == "1"
    with tile.TileContext(nc, trace_sim=trace_sim) as tc, ExitStack() as ctx:
        pool = ctx.enter_context(tc.tile_pool(name="main", bufs=1))
        stg = ctx.enter_context(tc.tile_pool(name="stg", bufs=3))
        tmp = ctx.enter_context(tc.tile_pool(name="tmp", bufs=3))
        psx = ctx.enter_context(tc.tile_pool(name="psx", bufs=2, space="PSUM"))
        psr_bufs = 1 if (with_gru and with_lstm) else 2
        psr = ctx.enter_context(tc.tile_pool(name="psr", bufs=psr_bufs, space="PSUM"))

        # ---- persistent SBUF tensors -----------------------------------
        mk_sb = pool.tile([128, T * 2 * b], U8)
        nc.sync.dma_start(mk_sb[:], mk[:])
        wo_sb = pool.tile([128, n_head_k], BF16)
        nc.sync.dma_start(wo_sb[:], wo[:])

        def load(p, shape, dt=BF16):
            t_ = pool.tile(shape, dt, tag=f"w_{p.name}")
            nc.sync.dma_start(t_[:], p[:])
            return t_

        if with_gru:
            gW0s = load(gW0, [D, 768])
            gU0s = load(gU0, [128, 2 * 768], WDT)
            gW1s = load(gW1, [128, 2 * 768], WDT)
            gU1s = load(gU1, [128, 2 * 768], WDT)
            bxgs = load(bxg, [128, NG_G], F32)
            bzr1s = load(bzr1, [128, 4 * b], F32)
            bxh1s = load(bxh1, [128, 2 * b], F32)
            brh1s = load(brh1, [128, 2 * b], F32)
            brh0s = load(brh0, [128, 2 * b], F32)
            gx_g = pool.tile([128, NG_G, TB], BF16)   # precomputed x-proj GRU L0
            o1 = pool.tile([128, 2, TB], BF16)        # GRU L1 output history
            hG0 = pool.tile([128, 2 * b], BF16)
            hG1 = pool.tile([128, 2 * b], BF16)
            nc.vector.memset(hG0[:], 0.0)
            nc.vector.memset(hG1[:], 0.0)
        if with_lstm:
            lW0s = load(lW0, [D, 1024])
            lU0s = load(lU0, [128, 2 * 1024], WDT)
            lW1s = load(lW1, [128, 2 * 1024], WDT)
            lU1s = load(lU1, [128, 2 * 1024], WDT)
            bxls = load(bxl, [128, NG_L], F32)
            bl1s = load(bl1, [128, NG_L * b], F32)
            gx_l = pool.tile([128, NG_L, TB], BF16)   # precomputed x-proj LSTM L0
            o2 = pool.tile([128, 2, TB], BF16)        # LSTM L1 output history
            hL0 = pool.tile([128, 2 * b], BF16)
            hL1 = pool.tile([128, 2 * b], BF16)
            cL0 = pool.tile([128, 2 * b], F32)
            cL1 = pool.tile([128, 2 * b], F32)
            for t_ in (hL0, hL1, cL0, cL1):
                nc.vector.memset(t_[:], 0.0)

        # ---- prologue: x-projections over all timesteps ----------------
        import os as _os
        _SKIP_PRO = _os.environ.get("K_SKIP_PRO", "0") == "1"
        _SKIP_EPI = _os.environ.get("K_SKIP_EPI", "0") == "1"
        _SKIP_LOOP = _os.environ.get("K_SKIP_LOOP", "0") == "1"
        if _SKIP_PRO:
            if with_gru:
                nc.vector.memset(gx_g[:], 0.0)
            if with_lstm:
                nc.vector.memset(gx_l[:], 0.0)
        for c in range(0 if _SKIP_PRO else TB // CH):
            xst = stg.tile([D, CH], BF16, tag="xst")
            nc.sync.dma_start(xst[:], xT[:, c * CH:(c + 1) * CH])
            if with_gru:
                for m in range(NG_G):
                    p = psx.tile([128, CH], F32, tag="px")
                    nc.tensor.matmul(p[:], gW0s[:, m * 128:(m + 1) * 128],
                                     xst[:], start=True, stop=True)
                    nc.vector.tensor_scalar(
                        gx_g[:, m, c * CH:(c + 1) * CH], p[:],
                        bxgs[:, m:m + 1], None, op0=OP.add)
            if with_lstm:
                for m in range(NG_L):
                    p = psx.tile([128, CH], F32, tag="px")
                    nc.tensor.matmul(p[:], lW0s[:, m * 128:(m + 1) * 128],
                                     xst[:], start=True, stop=True)
                    nc.vector.tensor_scalar(
                        gx_l[:, m, c * CH:(c + 1) * CH], p[:],
                        bxls[:, m:m + 1], None, op0=OP.add)

        # ---- recurrent loop --------------------------------------------
        def k2(w, k, m):
            """[128,128] lhsT slice: K-tile k, M-tile m of a [256, Mtot] weight."""
            mt = w.shape[1] // 2
            return w[:, k * mt + m * 128: k * mt + (m + 1) * 128]

        assert not fp8, "fp8 path disabled (accuracy)"

        def gru_math(p_zr, p_xh, p_hh, bias_zr, bias_xh, bias_rh, h, m_t, o_dst,
                     xh_from_psum=False):
            """p_*: PSUM APs; bias_* None -> skip."""
            if bias_zr is None:
                zr_in = p_zr
            else:
                szr = tmp.tile([128, 4 * b], F32, tag="szr")
                nc.vector.tensor_add(szr[:], p_zr, bias_zr)
                zr_in = szr[:]
            zr = tmp.tile([128, 4 * b], BF16, tag="zr")
            nc.scalar.activation(zr[:], zr_in, AF.Sigmoid)
            w_ = tmp.tile([128, 2 * b], F32, tag="w_")
            if bias_rh is None:
                nc.vector.tensor_mul(w_[:], zr[:, 2 * b:4 * b], p_hh)
            else:
                v = tmp.tile([128, 2 * b], F32, tag="v")
                nc.vector.tensor_add(v[:], p_hh, bias_rh)
                nc.vector.tensor_mul(w_[:], zr[:, 2 * b:4 * b], v[:])
            sh = tmp.tile([128, 2 * b], F32, tag="sh")
            if bias_xh is None:
                nc.vector.tensor_add(sh[:], w_[:], p_xh)
            else:
                sh2 = tmp.tile([128, 2 * b], F32, tag="sh2")
                nc.vector.tensor_add(sh2[:], p_xh, bias_xh)
                nc.vector.tensor_add(sh[:], w_[:], sh2[:])
            hh = tmp.tile([128, 2 * b], BF16, tag="hh")
            nc.scalar.activation(hh[:], sh[:], AF.Tanh)
            d = tmp.tile([128, 2 * b], BF16, tag="d")
            nc.vector.tensor_sub(d[:], h[:], hh[:])
            e = tmp.tile([128, 2 * b], BF16, tag="e")
            nc.vector.tensor_mul(e[:], zr[:, 0:2 * b], d[:])
            cand = tmp.tile([128, 2 * b], BF16, tag="cand")
            nc.vector.tensor_add(cand[:], hh[:], e[:])
            nc.vector.copy_predicated(h[:], m_t, cand[:])
            if o_dst is not None:
                nc.gpsimd.tensor_copy(o_dst, h[:])

        def lstm_math(p_g_full, gx_or_bias, h, c_, m_t, o_dst):
            if gx_or_bias is None:
                g = p_g_full
            else:
                gt = tmp.tile([128, NG_L * b], F32, tag="g")
                nc.vector.tensor_add(gt[:], p_g_full, gx_or_bias)
                g = gt[:]
            ifo = tmp.tile([128, 6 * b], BF16, tag="ifo")
            nc.scalar.activation(ifo[:], g[:, 0:6 * b], AF.Sigmoid)
            ct = tmp.tile([128, 2 * b], BF16, tag="ct")
            nc.scalar.activation(ct[:], g[:, 6 * b:8 * b], AF.Tanh)
            a1 = tmp.tile([128, 2 * b], F32, tag="a1")
            nc.vector.tensor_mul(a1[:], ifo[:, 2 * b:4 * b], c_[:])
            a2 = tmp.tile([128, 2 * b], F32, tag="a2")
            nc.vector.tensor_mul(a2[:], ifo[:, 0:2 * b], ct[:])
            cn = tmp.tile([128, 2 * b], F32, tag="cn")
            nc.vector.tensor_add(cn[:], a1[:], a2[:])
            nc.vector.copy_predicated(c_[:], m_t, cn[:])
            tch = tmp.tile([128, 2 * b], BF16, tag="tch")
            nc.scalar.activation(tch[:], c_[:], AF.Tanh)
            hc = tmp.tile([128, 2 * b], BF16, tag="hc")
            nc.vector.tensor_mul(hc[:], ifo[:, 4 * b:6 * b], tch[:])
            nc.vector.copy_predicated(h[:], m_t, hc[:])
            if o_dst is not None:
                nc.gpsimd.tensor_copy(o_dst, h[:])

        ORDER = os.environ.get("K_ORDER", "V1a")

        def step(off_b, off_2b):
            m_t = mk_sb[:, ds(off_2b, 2 * b)]
            if with_gru:
                pG0 = psr.tile([128, NG_G * b], F32, tag="pG0")
                # V0 regions: zr [0,4b) | xh [4b,6b) | hh [6b,8b)
                # V1 adds:    zrU [8b,12b)
                pG1 = psr.tile([128, 12 * b], F32, tag="pG1")
            if with_lstm:
                pL0 = psr.tile([128, NG_L * b], F32, tag="pL0")
                # V0: g [0,8b) accumulates U+W; V1: W [0,8b) | U [8b,16b)
                pL1 = psr.tile([128, 16 * b], F32, tag="pL1")
            uzr = ul1 = None

            # ---- U-side L1 products (need h1(t-1)) ----
            if with_gru:
                for m in range(4, 6):       # hh region: U only, complete group
                    for k in range(2):
                        nc.tensor.matmul(pG1[:, (m + 2) * b:(m + 3) * b],
                                         k2(gU1s, k, m), hG1[:, k * b:(k + 1) * b],
                                         start=(k == 0), stop=(k == 1))
                if ORDER == "V0":
                    for m in range(4):      # zr: U part opens the group
                        for k in range(2):
                            nc.tensor.matmul(pG1[:, m * b:(m + 1) * b],
                                             k2(gU1s, k, m), hG1[:, k * b:(k + 1) * b],
                                             start=(k == 0), stop=False)
                else:
                    for m in range(4):      # zrU: separate complete groups
                        for k in range(2):
                            nc.tensor.matmul(pG1[:, (8 + m) * b:(9 + m) * b],
                                             k2(gU1s, k, m), hG1[:, k * b:(k + 1) * b],
                                             start=(k == 0), stop=(k == 1))
            if with_lstm:
                off_u = 0 if ORDER == "V0" else 8
                for m in range(NG_L):
                    for k in range(2):
                        nc.tensor.matmul(pL1[:, (off_u + m) * b:(off_u + m + 1) * b],
                                         k2(lU1s, k, m), hL1[:, k * b:(k + 1) * b],
                                         start=(k == 0),
                                         stop=(k == 1 and ORDER != "V0"))
            if ORDER != "V0":
                act_evac = ORDER == "V1a"
                if with_gru:
                    uzr = tmp.tile([128, 4 * b], F32, tag="uzr")
                    if act_evac:
                        nc.scalar.copy(uzr[:], pG1[:, 8 * b:12 * b])
                    else:
                        nc.vector.tensor_copy(uzr[:], pG1[:, 8 * b:12 * b])
                if with_lstm:
                    ul1 = tmp.tile([128, 8 * b], F32, tag="ul1")
                    if act_evac:
                        nc.scalar.copy(ul1[:], pL1[:, 8 * b:16 * b])
                    else:
                        nc.vector.tensor_copy(ul1[:], pL1[:, 8 * b:16 * b])

            # ---- layer-0 recurrent matmuls ----
            if with_gru:
                for m in range(NG_G):
                    for k in range(2):
                        nc.tensor.matmul(pG0[:, m * b:(m + 1) * b],
                                         k2(gU0s, k, m), hG0[:, k * b:(k + 1) * b],
                                         start=(k == 0), stop=(k == 1))
            if with_lstm:
                for m in range(NG_L):
                    for k in range(2):
                        nc.tensor.matmul(pL0[:, m * b:(m + 1) * b],
                                         k2(lU0s, k, m), hL0[:, k * b:(k + 1) * b],
                                         start=(k == 0), stop=(k == 1))

            # ---- layer-0 gate math ----
            if with_gru:
                gru_math(pG0[:, 0:4 * b], gx_g[:, 4:6, ds(off_b, b)],
                         pG0[:, 4 * b:6 * b],
                         gx_g[:, 0:4, ds(off_b, b)], None,
                         None if zero_bias else brh0s[:],
                         hG0, m_t, None)
            if with_lstm:
                lstm_math(pL0[:], gx_l[:, :, ds(off_b, b)], hL0, cL0, m_t, None)

            # ---- W-side L1 products (need h0(t)) ----
            if with_gru:
                for m in range(4):
                    for k in range(2):
                        nc.tensor.matmul(pG1[:, m * b:(m + 1) * b],
                                         k2(gW1s, k, m), hG0[:, k * b:(k + 1) * b],
                                         start=(ORDER != "V0" and k == 0),
                                         stop=(k == 1))
                for m in range(4, 6):       # xh region: W only
                    for k in range(2):
                        nc.tensor.matmul(pG1[:, m * b:(m + 1) * b],
                                         k2(gW1s, k, m), hG0[:, k * b:(k + 1) * b],
                                         start=(k == 0), stop=(k == 1))
            if with_lstm:
                for m in range(NG_L):
                    for k in range(2):
                        nc.tensor.matmul(pL1[:, m * b:(m + 1) * b],
                                         k2(lW1s, k, m), hL0[:, k * b:(k + 1) * b],
                                         start=(ORDER != "V0" and k == 0),
                                         stop=(k == 1))

            # ---- layer-1 gate math ----
            if with_gru:
                if ORDER == "V0":
                    p_zr = pG1[:, 0:4 * b]
                else:
                    szrl1 = tmp.tile([128, 4 * b], F32, tag="szrl1")
                    nc.vector.tensor_add(szrl1[:], uzr[:], pG1[:, 0:4 * b])
                    p_zr = szrl1[:]
                gru_math(p_zr, pG1[:, 4 * b:6 * b], pG1[:, 6 * b:8 * b],
                         None if zero_bias else bzr1s[:],
                         None if zero_bias else bxh1s[:],
                         None if zero_bias else brh1s[:],
                         hG1, m_t, o1[:, :, ds(off_b, b)])
            if with_lstm:
                if ORDER == "V0":
                    p_g = pL1[:, 0:8 * b]
                else:
                    gl1 = tmp.tile([128, 8 * b], F32, tag="gl1")
                    nc.vector.tensor_add(gl1[:], ul1[:], pL1[:, 0:8 * b])
                    p_g = gl1[:]
                lstm_math(p_g, None if zero_bias else bl1s[:], hL1, cL1, m_t,
                          o2[:, :, ds(off_b, b)])


        # ---- pipelined GRU-only body: interleave G1-math(t-1) x G0-math(t)
        def gru_mm_l0(pG0, h0):
            for m in range(NG_G):
                for k in range(2):
                    nc.tensor.matmul(pG0[:, m * b:(m + 1) * b],
                                     k2(gU0s, k, m), h0[:, k * b:(k + 1) * b],
                                     start=(k == 0), stop=(k == 1))

        def gru_mm_u(pG1, h1):
            for m in range(4, 6):
                for k in range(2):
                    nc.tensor.matmul(pG1[:, (m + 2) * b:(m + 3) * b],
                                     k2(gU1s, k, m), h1[:, k * b:(k + 1) * b],
                                     start=(k == 0), stop=(k == 1))
            for m in range(4):
                for k in range(2):
                    nc.tensor.matmul(pG1[:, (8 + m) * b:(9 + m) * b],
                                     k2(gU1s, k, m), h1[:, k * b:(k + 1) * b],
                                     start=(k == 0), stop=(k == 1))

        def gru_mm_w(pG1, h0):
            for m in range(4):
                for k in range(2):
                    nc.tensor.matmul(pG1[:, m * b:(m + 1) * b],
                                     k2(gW1s, k, m), h0[:, k * b:(k + 1) * b],
                                     start=(k == 0), stop=(k == 1))
            for m in range(4, 6):
                for k in range(2):
                    nc.tensor.matmul(pG1[:, m * b:(m + 1) * b],
                                     k2(gW1s, k, m), h0[:, k * b:(k + 1) * b],
                                     start=(k == 0), stop=(k == 1))

        class GChain:
            """One gate-math chain (either layer), emitted in stages."""
            def __init__(self, p_zr, p_xh, p_hh, xh_sbuf, h, m_t, o_dst):
                self.p_zr, self.p_xh, self.p_hh = p_zr, p_xh, p_hh
                self.xh_sbuf = xh_sbuf
                self.h, self.m_t, self.o_dst = h, m_t, o_dst

            def s_sigma(self):
                self.zr = tmp.tile([128, 4 * b], BF16, tag="zr")
                nc.scalar.activation(self.zr[:], self.p_zr, AF.Sigmoid)

            def s_wsh(self):
                self.sh = tmp.tile([128, 2 * b], F32, tag="sh")
                w_ = tmp.tile([128, 2 * b], F32, tag="w_")
                nc.vector.tensor_mul(w_[:], self.zr[:, 2 * b:4 * b], self.p_hh)
                nc.vector.tensor_add(self.sh[:], w_[:], self.p_xh)

            def s_tanh(self):
                self.hh = tmp.tile([128, 2 * b], BF16, tag="hh")
                nc.scalar.activation(self.hh[:], self.sh[:], AF.Tanh)

            def s_update(self):
                d = tmp.tile([128, 2 * b], BF16, tag="d")
                nc.vector.tensor_sub(d[:], self.h[:], self.hh[:])
                e = tmp.tile([128, 2 * b], BF16, tag="e")
                nc.vector.tensor_mul(e[:], self.zr[:, 0:2 * b], d[:])
                cand = tmp.tile([128, 2 * b], BF16, tag="cand")
                nc.vector.tensor_add(cand[:], self.hh[:], e[:])
                nc.vector.copy_predicated(self.h[:], self.m_t, cand[:])
                if self.o_dst is not None:
                    nc.gpsimd.tensor_copy(self.o_dst, self.h[:])

        def gru_pipelined_body(i):
            prev = None          # G1 chain of previous j
            prev_pG1 = None
            prev_uzr = None
            for j in range(unroll):
                off_b = i * (unroll * b) + j * b
                off_2b = i * (unroll * 2 * b) + j * 2 * b
                m_t = mk_sb[:, ds(off_2b, 2 * b)]
                pG0 = psr.tile([128, NG_G * b], F32, tag="pG0")
                pG1 = psr.tile([128, 12 * b], F32, tag="pG1")
                gru_mm_l0(pG0, hG0)
                if prev is not None:
                    gru_mm_w(prev_pG1, hG0)
                gru_mm_u(pG1, hG1)
                # assemble G0 chain for this step
                szr = tmp.tile([128, 4 * b], F32, tag="szr")
                nc.vector.tensor_add(szr[:], pG0[:, 0:4 * b],
                                     gx_g[:, 0:4, ds(off_b, b)])
                g0 = GChain(szr[:], gx_g[:, 4:6, ds(off_b, b)],
                            pG0[:, 4 * b:6 * b], True, hG0, m_t, None)
                seqmode = os.environ.get("K_GRUPIPE", "1") == "2"
                if prev is not None:
                    szrl1 = tmp.tile([128, 4 * b], F32, tag="szrl1")
                    nc.vector.tensor_add(szrl1[:], prev_uzr[:],
                                         prev_pG1[:, 0:4 * b])
                    prev.p_zr = szrl1[:]
                    if seqmode:
                        prev.s_sigma(); prev.s_wsh(); prev.s_tanh(); prev.s_update()
                    else:
                        prev.s_sigma()
                if not seqmode and prev is not None:
                    g0.s_sigma()
                    prev.s_wsh()
                    g0.s_wsh()
                    prev.s_tanh()
                    g0.s_tanh()
                    prev.s_update()
                    g0.s_update()
                else:
                    g0.s_sigma(); g0.s_wsh(); g0.s_tanh(); g0.s_update()
                uzr = tmp.tile([128, 4 * b], F32, tag="uzr")
                nc.vector.tensor_copy(uzr[:], pG1[:, 8 * b:12 * b])
                prev = GChain(None, pG1[:, 4 * b:6 * b], pG1[:, 6 * b:8 * b],
                              False, hG1, m_t, o1[:, :, ds(off_b, b)])
                prev_pG1, prev_uzr = pG1, uzr
            # tail: finish G1 chain of the last step in this body
            gru_mm_w(prev_pG1, hG0)
            szrl1 = tmp.tile([128, 4 * b], F32, tag="szrl1")
            nc.vector.tensor_add(szrl1[:], prev_uzr[:], prev_pG1[:, 0:4 * b])
            prev.p_zr = szrl1[:]
            prev.s_sigma(); prev.s_wsh(); prev.s_tanh(); prev.s_update()

        use_pipe = (with_gru and not with_lstm and zero_bias
                    and os.environ.get("K_GRUPIPE", "0") in ("1", "2"))

        n_outer = T // unroll
        hint = tuple()
        if os.environ.get("K_HINT", "0") == "1":
            hint = (mybir.EngineType.PE, mybir.EngineType.DVE,
                    mybir.EngineType.Activation)
        stag = os.environ.get("K_STAG", "0") == "1"
        if not _SKIP_LOOP:
            if time_mult == 1:
                with tc.For_i(0, n_outer, 1, hint_engines=hint,
                              staggered_reset=stag) as i:
                    if use_pipe:
                        gru_pipelined_body(i)
                    else:
                        for j in range(unroll):
                            step(i * (unroll * b) + j * b,
                                 i * (unroll * 2 * b) + j * 2 * b)
            else:
                with tc.For_i(0, time_mult, 1) as _rep:
                    with tc.For_i(0, n_outer, 1, hint_engines=hint,
                                  staggered_reset=stag) as i:
                        if use_pipe:
                            gru_pipelined_body(i)
                        else:
                            for j in range(unroll):
                                step(i * (unroll * b) + j * b,
                                     i * (unroll * 2 * b) + j * 2 * b)
        else:
            if with_gru:
                nc.vector.memset(o1[:], 0.0)
            if with_lstm:
                nc.vector.memset(o2[:], 0.0)

        # ---- epilogue: dense head --------------------------------------
        if _SKIP_EPI:
            y_sb0 = tmp.tile([1, CH], F32, tag="ych")
            nc.vector.memset(y_sb0[:], 0.0)
            nc.sync.dma_start(y_ext[:, 0:CH], y_sb0[:])
        for c in range(0 if _SKIP_EPI else TB // CH):
            py = psx.tile([1, CH], F32, tag="py")
            srcs = []
            if with_gru:
                srcs += [o1[:, 0, c * CH:(c + 1) * CH], o1[:, 1, c * CH:(c + 1) * CH]]
            if with_lstm:
                srcs += [o2[:, 0, c * CH:(c + 1) * CH], o2[:, 1, c * CH:(c + 1) * CH]]
            for ki, s in enumerate(srcs):
                nc.tensor.matmul(py[:], wo_sb[:, ki:ki + 1], s,
                                 start=(ki == 0), stop=(ki == len(srcs) - 1))
            ych = tmp.tile([1, CH], F32, tag="ych")
            nc.vector.tensor_copy(ych[:], py[:])
            nc.sync.dma_start(y_ext[:, c * CH:(c + 1) * CH], ych[:])

    nc.finalize()
    return nc


def build_nc2(b, model, zero_bias=False, chunk=None, fp8=False):
    """v2: chunked software pipeline. L1 runs one chunk (C steps) behind L0;
    the L1 input projections W1 @ h0 are computed as bulk matmuls (free dim
    C*b=128) once per chunk instead of per-step, cutting the per-step PE
    instruction count from 48 to 32+2 (LSTM) / 36 to 24+1.5 (GRU). The PE
    pair cost is ~flat in free-dim (N=16 ~92ns vs N=128 ~110ns), so bulk
    W1 work is nearly free.

    model: "gru" | "lstm". Single-model builds only (p2 dispatch).
    """
    C = chunk or UNROLL
    nc = bacc.Bacc()
    TB = T * b
    with_gru = model == "gru"
    NG = NG_G if with_gru else NG_L
    n_chunks = T // C
    assert n_chunks % 2 == 0 and n_chunks >= 4
    WDT = FP8 if fp8 else BF16
    SC = 1.0 / WSCALE if fp8 else 1.0   # descale on pre-activation sums
    DR = mybir.MatmulPerfMode.DoubleRow if fp8 else None

    xT = nc.declare_dram_parameter("xT", [D, TB], BF16, isOutput=False)
    mk = nc.declare_dram_parameter("mask", [128, T * 2 * b], U8, isOutput=False)
    y_ext = nc.declare_dram_parameter("y", [1, TB], F32, isOutput=True)

    def param(name, shape, dt=BF16):
        return nc.declare_dram_parameter(name, shape, dt, isOutput=False)

    if with_gru:
        W0 = param("gW0", [D, 768])
        U0 = param("gU0", [128, 2 * 768], WDT)
        W1 = param("gW1", [128, 2 * 768], WDT)
        U1 = param("gU1", [128, 2 * 768], WDT)
        bx = param("bxg", [128, NG], F32)
        bzr1 = param("bzr1", [128, 4 * b], F32)
        bxh1 = param("bxh1", [128, 2 * b], F32)
        brh1 = param("brh1", [128, 2 * b], F32)
        brh0 = param("brh0", [128, 2 * b], F32)
    else:
        W0 = param("lW0", [D, 1024])
        U0 = param("lU0", [128, 2 * 1024], WDT)
        W1 = param("lW1", [128, 2 * 1024], WDT)
        U1 = param("lU1", [128, 2 * 1024], WDT)
        bx = param("bxl", [128, NG], F32)
        bl1 = param("bl1", [128, NG * b], F32)
    wo = param("wo", [128, 2])

    trace_sim = os.environ.get("K_TRACE", "0") == "1"
    with tile.TileContext(nc, trace_sim=trace_sim) as tc, ExitStack() as ctx:
        pool = ctx.enter_context(tc.tile_pool(name="main", bufs=1))
        stg = ctx.enter_context(tc.tile_pool(name="stg", bufs=2))
        tmp = ctx.enter_context(tc.tile_pool(name="tmp", bufs=2))
        psx = ctx.enter_context(tc.tile_pool(name="psx", bufs=1, space="PSUM"))
        psr = ctx.enter_context(tc.tile_pool(name="psr", bufs=2, space="PSUM"))
        psw = ctx.enter_context(tc.tile_pool(name="psw", bufs=1, space="PSUM"))

        mk_sb = pool.tile([128, T * 2 * b], U8)
        nc.sync.dma_start(mk_sb[:], mk[:])
        wo_sb = pool.tile([128, 2], BF16)
        nc.sync.dma_start(wo_sb[:], wo[:])

        def load(p, shape, dt=BF16):
            t_ = pool.tile(shape, dt, tag=f"w_{p.name}")
            nc.sync.dma_start(t_[:], p[:])
            return t_

        W0s = load(W0, [D, NG * 128])
        wshape = [128, 2, NG * 128] if fp8 else [128, 2 * NG * 128]
        U0s = load(U0, wshape, WDT)
        W1s = load(W1, wshape, WDT)
        U1s = load(U1, wshape, WDT)
        bxs = load(bx, [128, NG], F32)
        if with_gru:
            bzr1s = load(bzr1, [128, 4 * b], F32)
            bxh1s = load(bxh1, [128, 2 * b], F32)
            brh1s = load(brh1, [128, 2 * b], F32)
            brh0s = load(brh0, [128, 2 * b], F32)
        else:
            bl1s = load(bl1, [128, NG * b], F32)

        gx = pool.tile([128, NG, TB], BF16)       # L0 x-proj, all timesteps
        oh = pool.tile([128, 2, TB], BF16)        # L1 output history (head in)
        h0 = pool.tile([128, 2 * b], BF16)
        h1 = pool.tile([128, 2 * b], BF16)
        nc.vector.memset(h0[:], 0.0)
        nc.vector.memset(h1[:], 0.0)
        if not with_gru:
            c0 = pool.tile([128, 2 * b], F32)
            c1 = pool.tile([128, 2 * b], F32)
            nc.vector.memset(c0[:], 0.0)
            nc.vector.memset(c1[:], 0.0)
        hist0 = pool.tile([128, 2, C * b], WDT, tag="hist0")
        hist1 = pool.tile([128, 2, C * b], WDT, tag="hist1")
        if fp8:
            h80 = pool.tile([128, 2, b], FP8, tag="h80")
            h81 = pool.tile([128, 2, b], FP8, tag="h81")
            nc.vector.memset(h80[:], 0.0)
            nc.vector.memset(h81[:], 0.0)
        w1ev0 = pool.tile([128, NG, C * b], BF16, tag="w1ev0")
        w1ev1 = pool.tile([128, NG, C * b], BF16, tag="w1ev1")
        hist = [hist0, hist1]
        w1ev = [w1ev0, w1ev1]

        # ---- prologue: x-projections over all timesteps ----------------
        for cc in range(TB // CH):
            xst = stg.tile([D, CH], BF16, tag="xst")
            nc.sync.dma_start(xst[:], xT[:, cc * CH:(cc + 1) * CH])
            for m in range(NG):
                p = psx.tile([128, CH], F32, tag="px")
                nc.tensor.matmul(p[:], W0s[:, m * 128:(m + 1) * 128],
                                 xst[:], start=True, stop=True)
                if fp8:
                    nc.scalar.activation(
                        gx[:, m, cc * CH:(cc + 1) * CH], p[:], AF.Copy,
                        bias=bxs[:, m:m + 1], scale=WSCALE)
                else:
                    nc.vector.tensor_scalar(
                        gx[:, m, cc * CH:(cc + 1) * CH], p[:],
                        bxs[:, m:m + 1], None, op0=OP.add)

        def k2(w, k, m):
            mt = w.shape[1] // 2
            return w[:, k * mt + m * 128: k * mt + (m + 1) * 128]

        def mm_rec(ptile, Ws, hsrc):
            for m in range(NG):
                for k in range(2):
                    nc.tensor.matmul(ptile[:, m * b:(m + 1) * b],
                                     k2(Ws, k, m), hsrc[:, k * b:(k + 1) * b],
                                     start=(k == 0), stop=(k == 1))

        def gru_cell(szr, xh_src, p_hh, bias_rh, h, m_t, hist_dst):
            """szr: [128,4b] f32 pre-activation for z,r (already summed).
            xh_src: [128,(2),b] additive candidate input (x/W1 side).
            p_hh: [128,2b] recurrent candidate part (U side, pre-bias)."""
            zr = tmp.tile([128, 4 * b], BF16, tag="zr")
            nc.scalar.activation(zr[:], szr, AF.Sigmoid)
            w_ = tmp.tile([128, 2 * b], F32, tag="w_")
            if bias_rh is None:
                nc.vector.tensor_mul(w_[:], zr[:, 2 * b:4 * b], p_hh)
            else:
                v = tmp.tile([128, 2 * b], F32, tag="v")
                nc.vector.tensor_add(v[:], p_hh, bias_rh)
                nc.vector.tensor_mul(w_[:], zr[:, 2 * b:4 * b], v[:])
            sh = tmp.tile([128, 2 * b], F32, tag="sh")
            nc.vector.tensor_add(sh[:], w_[:], xh_src)
            hh = tmp.tile([128, 2 * b], BF16, tag="hh")
            nc.scalar.activation(hh[:], sh[:], AF.Tanh)
            d = tmp.tile([128, 2 * b], BF16, tag="d")
            nc.vector.tensor_sub(d[:], h[:], hh[:])
            e = tmp.tile([128, 2 * b], BF16, tag="e")
            nc.vector.tensor_mul(e[:], zr[:, 0:2 * b], d[:])
            cand = tmp.tile([128, 2 * b], BF16, tag="cand")
            nc.vector.tensor_add(cand[:], hh[:], e[:])
            nc.vector.copy_predicated(h[:], m_t, cand[:])
            if hist_dst is not None:
                nc.gpsimd.tensor_copy(hist_dst, h[:])

        def lstm_cell(g, h, c_, m_t, hist_dst):
            ifo = tmp.tile([128, 6 * b], BF16, tag="ifo")
            nc.scalar.activation(ifo[:], g[:, 0:6 * b], AF.Sigmoid)
            ct = tmp.tile([128, 2 * b], BF16, tag="ct")
            nc.scalar.activation(ct[:], g[:, 6 * b:8 * b], AF.Tanh)
            a1 = tmp.tile([128, 2 * b], F32, tag="a1")
            nc.vector.tensor_mul(a1[:], ifo[:, 2 * b:4 * b], c_[:])
            a2 = tmp.tile([128, 2 * b], F32, tag="a2")
            nc.vector.tensor_mul(a2[:], ifo[:, 0:2 * b], ct[:])
            cn = tmp.tile([128, 2 * b], F32, tag="cn")
            nc.vector.tensor_add(cn[:], a1[:], a2[:])
            nc.vector.copy_predicated(c_[:], m_t, cn[:])
            tch = tmp.tile([128, 2 * b], BF16, tag="tch")
            nc.scalar.activation(tch[:], c_[:], AF.Tanh)
            hc = tmp.tile([128, 2 * b], BF16, tag="hc")
            nc.vector.tensor_mul(hc[:], ifo[:, 4 * b:6 * b], tch[:])
            nc.vector.copy_predicated(h[:], m_t, hc[:])
            if hist_dst is not None:
                nc.gpsimd.tensor_copy(hist_dst, h[:])

        def a_step(off_b, off_2b, par, j):
            """L0 step t; h0 recurrence + history write for bulk W1."""
            m_t = mk_sb[:, ds(off_2b, 2 * b)]
            pA = psr.tile([128, NG * b], F32, tag="pA")
            mm_rec(pA, U0s, h0)
            hd = hist[par][:, :, j * b:(j + 1) * b]
            if with_gru:
                szr = tmp.tile([128, 4 * b], F32, tag="szrA")
                nc.vector.tensor_add(szr[:], pA[:, 0:4 * b],
                                     gx[:, 0:4, ds(off_b, b)])
                gru_cell(szr[:], gx[:, 4:6, ds(off_b, b)], pA[:, 4 * b:6 * b],
                         None if zero_bias else brh0s[:], h0, m_t, hd)
            else:
                gt = tmp.tile([128, NG * b], F32, tag="gtA")
                nc.vector.tensor_add(gt[:], pA[:], gx[:, :, ds(off_b, b)])
                lstm_cell(gt, h0, c0, m_t, hd)

        def c_step(off_b, off_2b, par, j):
            """L1 step t; reads bulk W1 products of its chunk (parity par)."""
            m_t = mk_sb[:, ds(off_2b, 2 * b)]
            pC = psr.tile([128, NG * b], F32, tag="pC")
            mm_rec(pC, U1s, h1)
            od = oh[:, :, ds(off_b, b)]
            wj = w1ev[par][:, :, j * b:(j + 1) * b]
            if with_gru:
                szr = tmp.tile([128, 4 * b], F32, tag="szrC")
                nc.vector.tensor_add(szr[:], pC[:, 0:4 * b],
                                     w1ev[par][:, 0:4, j * b:(j + 1) * b])
                if not zero_bias:
                    szr2 = tmp.tile([128, 4 * b], F32, tag="szrC2")
                    nc.vector.tensor_add(szr2[:], szr[:], bzr1s[:])
                    szr = szr2
                xh = w1ev[par][:, 4:6, j * b:(j + 1) * b]
                if not zero_bias:
                    xh2 = tmp.tile([128, 2 * b], F32, tag="xhC")
                    nc.vector.tensor_add(xh2[:], xh, bxh1s[:])
                    xh = xh2[:]
                gru_cell(szr[:], xh, pC[:, 4 * b:6 * b],
                         None if zero_bias else brh1s[:], h1, m_t, od)
            else:
                gt = tmp.tile([128, NG * b], F32, tag="gtC")
                nc.vector.tensor_add(gt[:], pC[:], wj)
                if not zero_bias:
                    gt2 = tmp.tile([128, NG * b], F32, tag="gtC2")
                    nc.vector.tensor_add(gt2[:], gt[:], bl1s[:])
                    gt = gt2
                lstm_cell(gt, h1, c1, m_t, od)

        def bulk_w1(par):
            pW = psw.tile([128, NG, C * b], F32, tag="pW")
            for m in range(NG):
                for k in range(2):
                    nc.tensor.matmul(pW[:, m, :], k2(W1s, k, m),
                                     hist[par][:, k, :],
                                     start=(k == 0), stop=(k == 1))
            nc.scalar.copy(w1ev[par][:], pW[:])

        def half(base_b, base_2b, k_par):
            """A-steps of chunk k (parity k_par) + C-steps of chunk k-1,
            then bulk W1 of chunk k. base_* = element offsets of chunk k."""
            for j in range(C):
                a_step(base_b + j * b, base_2b + j * 2 * b, k_par, j)
                c_step(base_b - C * b + j * b, base_2b - 2 * C * b + j * 2 * b,
                       1 - k_par, j)
            bulk_w1(k_par)

        # ---- prologue chunk 0 ------------------------------------------
        for j in range(C):
            a_step(j * b, j * 2 * b, 0, j)
        bulk_w1(0)

        # ---- main loop: halves k=2i+1, k=2i+2 --------------------------
        n_outer = (n_chunks - 2) // 2
        with tc.For_i(0, n_outer, 1) as i:
            half(i * (2 * C * b) + C * b, i * (4 * C * b) + 2 * C * b, 1)
            half(i * (2 * C * b) + 2 * C * b, i * (4 * C * b) + 4 * C * b, 0)

        # ---- epilogue: chunk 63 + final C chunk ------------------------
        last = n_chunks - 1
        half(last * C * b, last * 2 * C * b, last % 2)
        for j in range(C):
            c_step(last * C * b + j * b, last * 2 * C * b + j * 2 * b,
                   last % 2, j)

        # ---- dense head ------------------------------------------------
        for cc in range(TB // CH):
            py = psx.tile([1, CH], F32, tag="py")
            srcs = [oh[:, 0, cc * CH:(cc + 1) * CH],
                    oh[:, 1, cc * CH:(cc + 1) * CH]]
            for ki, s in enumerate(srcs):
                nc.tensor.matmul(py[:], wo_sb[:, ki:ki + 1], s,
                                 start=(ki == 0), stop=(ki == len(srcs) - 1))
            ych = tmp.tile([1, CH], F32, tag="ych")
            nc.vector.tensor_copy(ych[:], py[:])
            nc.sync.dma_start(y_ext[:, cc * CH:(cc + 1) * CH], ych[:])

    nc.finalize()
    return nc


# ---------------------------------------------------------------------------
# Host-side data preparation
# ---------------------------------------------------------------------------

def _fold_k(w):
    """[256, M] -> [128, 2*M] with K-tile-major free layout."""
    m = w.shape[1]
    return np.ascontiguousarray(
        w.reshape(2, 128, m).transpose(1, 0, 2).reshape(128, 2 * m))


def _ptile(v):
    """[n*128] vector -> [128, n] per-partition tile layout."""
    n = v.shape[0] // 128
    return np.ascontiguousarray(v.reshape(n, 128).T)


def _bcast_b(v2, b):
    """[128, n] -> [128, n*b] replicated along batch."""
    return np.ascontiguousarray(
        np.repeat(v2[:, :, None], b, axis=2).reshape(128, -1))


_LSTM_PERM = np.r_[0:512, 768:1024, 512:768]   # i,f,c,o -> i,f,o,c


def prep_shared(inputs, b, with_gru=True, with_lstm=True, fp8=False):
    f32 = np.float32
    bf = ml_dtypes.bfloat16
    wdt = ml_dtypes.float8_e4m3 if fp8 else bf
    wsc = WSCALE if fp8 else 1.0
    out = {}
    if with_gru:
        bi0, br0 = inputs["g_bi0"].astype(f32), inputs["g_br0"].astype(f32)
        bx = np.concatenate([(bi0 + br0)[:512], bi0[512:]])
        out["gW0"] = np.ascontiguousarray(inputs["g_W0"]).astype(bf)
        out["gU0"] = (_fold_k(inputs["g_U0"].astype(f32)) * wsc).astype(wdt)
        out["gW1"] = (_fold_k(inputs["g_W1"].astype(f32)) * wsc).astype(wdt)
        out["gU1"] = (_fold_k(inputs["g_U1"].astype(f32)) * wsc).astype(wdt)
        out["bxg"] = _ptile(bx)
        bi1, br1 = inputs["g_bi1"].astype(f32), inputs["g_br1"].astype(f32)
        out["bzr1"] = _bcast_b(_ptile((bi1 + br1)[:512]), b)
        out["bxh1"] = _bcast_b(_ptile(bi1[512:]), b)
        out["brh1"] = _bcast_b(_ptile(br1[512:]), b)
        out["brh0"] = _bcast_b(_ptile(br0[512:]), b)
    if with_lstm:
        out["lW0"] = np.ascontiguousarray(
            inputs["l_W0"][:, _LSTM_PERM]).astype(bf)
        out["lU0"] = (_fold_k(inputs["l_U0"][:, _LSTM_PERM].astype(f32)) * wsc).astype(wdt)
        out["lW1"] = (_fold_k(inputs["l_W1"][:, _LSTM_PERM].astype(f32)) * wsc).astype(wdt)
        out["lU1"] = (_fold_k(inputs["l_U1"][:, _LSTM_PERM].astype(f32)) * wsc).astype(wdt)
        out["bxl"] = _ptile(inputs["l_b0"][_LSTM_PERM].astype(f32))
        out["bl1"] = _bcast_b(_ptile(inputs["l_b1"][_LSTM_PERM].astype(f32)), b)
    w = inputs["out_W"][:, 0].astype(f32)        # [512]
    cols = []
    if with_gru:
        cols += [w[0:128], w[128:256]]
    if with_lstm:
        cols += [w[256:384], w[384:512]]
    out["wo"] = np.stack(cols, axis=1).astype(bf)
    return out


def prep_core(inputs, idx, b):
    """Per-core x transpose + mask broadcast for batch indices idx."""
    bf = ml_dtypes.bfloat16
    x = inputs["x"][idx]                          # [b, T, D]
    xT = np.ascontiguousarray(x.transpose(2, 1, 0).reshape(D, T * b)).astype(bf)
    lens = np.asarray(inputs["lengths"])[idx]
    m = (np.arange(T)[:, None] < lens[None, :])   # [T, b]
    mk = np.broadcast_to(m[None, :, None, :], (128, T, 2, b))
    mk = np.ascontiguousarray(mk.reshape(128, T * 2 * b)).astype(np.uint8)
    return {"xT": xT, "mask": mk}


_NC_CACHE = {}


def make_exec(nc, dev_off, n_cores):
    """Compile nc into a jitted shard_map over devices[dev_off:dev_off+n_cores].

    Returns (dispatch, finish): dispatch(in_maps) launches asynchronously and
    returns a handle; finish(handle) blocks and returns per-core result dicts.
    """
    import jax
    from jax.sharding import Mesh, PartitionSpec
    from jax.experimental.shard_map import shard_map
    from concourse.bass2jax import (_bass_exec_p, install_neuronx_cc_hook,
                                    partition_id_tensor)

    install_neuronx_cc_hook()
    partition_name = nc.partition_id_tensor.name if nc.partition_id_tensor else None
    in_names, out_names, out_avals = [], [], []
    for alloc in nc.m.functions[0].allocations:
        if not isinstance(alloc, mybir.MemoryLocationSet):
            continue
        name = alloc.memorylocations[0].name
        if alloc.kind == "ExternalInput":
            if name != partition_name:
                in_names.append(name)
        elif alloc.kind == "ExternalOutput":
            out_names.append(name)
            out_avals.append(jax.core.ShapedArray(
                tuple(alloc.tensor_shape), mybir.dt.np(alloc.dtype)))
    n_params = len(in_names)
    all_in = list(in_names) + list(out_names)
    if partition_name is not None:
        all_in.append(partition_name)

    def _body(*args):
        operands = list(args)
        if partition_name is not None:
            operands.append(partition_id_tensor())
        return tuple(_bass_exec_p.bind(
            *operands, out_avals=tuple(out_avals), in_names=tuple(all_in),
            out_names=tuple(out_names), lowering_input_output_aliases=(),
            sim_require_finite=True, sim_require_nnan=True, nc=nc))

    devices = jax.devices()[dev_off:dev_off + n_cores]
    mesh = Mesh(np.asarray(devices), ("core",))
    nio = n_params + len(out_avals)
    sharded = jax.jit(shard_map(_body, mesh=mesh,
                                in_specs=(PartitionSpec("core"),) * nio,
                                out_specs=(PartitionSpec("core"),) * len(out_avals),
                                check_rep=False), keep_unused=True)

    def dispatch(in_maps):
        concat_in = [np.concatenate([np.asarray(in_maps[c][n])
                                     for c in range(n_cores)], axis=0)
                     for n in in_names]
        zeros = [np.zeros((n_cores * a.shape[0], *a.shape[1:]), a.dtype)
                 for a in out_avals]
        return sharded(*concat_in, *zeros)

    def finish(handle):
        import jax
        jax.block_until_ready(handle)
        return [{n: np.asarray(handle[i]).reshape(n_cores, *out_avals[i].shape)[c]
                 for i, n in enumerate(out_names)} for c in range(n_cores)]

    dispatch.sharded = sharded
    dispatch.in_names = in_names
    dispatch.out_avals = out_avals
    dispatch.mesh = mesh
    return dispatch, finish


MODE = os.environ.get("K_MODE", "p2")


def _all_bias_zero(inputs):
    return all(not np.any(np.asarray(inputs[k]))
               for k in ("g_bi0", "g_br0", "g_bi1", "g_br1", "l_b0", "l_b1"))


FP8_ON = os.environ.get("K_FP8", "0") == "1"


K_IMPL = os.environ.get("K_IMPL", "v2")


def _get_p2(zb):
    key = ("p2", zb, FP8_ON, K_IMPL)
    if key not in _NC_CACHE:
        bl = B // (NCORES // 2)      # 16 examples per core
        if K_IMPL == "v2":
            nc_g = build_nc2(bl, "gru", zero_bias=zb)
            nc_l = build_nc2(bl, "lstm", zero_bias=zb)
        else:
            nc_g = build_nc(bl, True, False, zero_bias=zb, fp8=FP8_ON)
            nc_l = build_nc(bl, False, True, zero_bias=zb, fp8=FP8_ON)
        dg, fg = make_exec(nc_g, 0, NCORES // 2)
        dl, fl = make_exec(nc_l, NCORES // 2, NCORES // 2)
        _NC_CACHE[key] = (nc_g, nc_l, dg, fg, dl, fl, bl)
    return _NC_CACHE[key]


def kernel(**inputs) -> np.ndarray:
    out_b = float(np.asarray(inputs["out_b"]).reshape(-1)[0])
    y = np.empty((B, T, 1), np.float32)
    if MODE == "p1":
        if "p1" not in _NC_CACHE:
            _NC_CACHE["p1"] = build_nc(B_LOC, True, True)
        nc = _NC_CACHE["p1"]
        shared = prep_shared(inputs, B_LOC, True, True)
        in_maps = []
        for c in range(NCORES):
            m = dict(shared)
            m.update(prep_core(inputs, slice(c * B_LOC, (c + 1) * B_LOC), B_LOC))
            in_maps.append(m)
        res = run_bass_kernel_spmd(nc, in_maps, core_ids=list(range(NCORES)))
        for c in range(NCORES):
            yc = res.results[c]["y"].reshape(T, B_LOC)
            y[c * B_LOC:(c + 1) * B_LOC, :, 0] = yc.T + out_b
        return y

    nc_g, nc_l, dg, fg, dl, fl, bl = _get_p2(_all_bias_zero(inputs))
    hc = NCORES // 2
    sh_g = prep_shared(inputs, bl, True, False, fp8=FP8_ON)
    sh_l = prep_shared(inputs, bl, False, True, fp8=FP8_ON)
    maps_g, maps_l = [], []
    for c in range(hc):
        core = prep_core(inputs, slice(c * bl, (c + 1) * bl), bl)
        mg = dict(sh_g); mg.update(core); maps_g.append(mg)
        ml = dict(sh_l); ml.update(core); maps_l.append(ml)
    hg = dg(maps_g)
    hl = dl(maps_l)
    res_g = fg(hg)
    res_l = fl(hl)
    for c in range(hc):
        yc = (res_g[c]["y"].astype(np.float32)
              + res_l[c]["y"].astype(np.float32)).reshape(T, bl)
        y[c * bl:(c + 1) * bl, :, 0] = yc.T + out_b
    return y

